# revision 38
# baseline (speedup 1.0000x reference)
"""NePuEncoder Bass/Tile kernel for 8 Trainium2 NeuronCores.

Sharding: query-parallel. Core c handles batch b=c//4, queries qo=(c%4)*96 ..
qo+96 of that batch. Channel-major layout [128 chan, keys] throughout.

Per-query attention fully fused in SBUF/PSUM:
  - PE:   hpre = G1P8(fp8,K=32)@trig8 + Wg1(bf16)@EK;  logits = Wg2@h;
          pos  = ONE fp8 DoubleRow matmul (I8 (x) VK8  +  PsShift8 (x) trig8)
          (per-query PE cost 560ns vs 800ns all-bf16)
  - relu(hpre+QB[m]): 1/3 on ACT, 2/3 on DVE (balance)
  - ACT:  w = exp(logits + bg2) with accum -> S0
  - DVE:  S1[m] = sum_n (pos + QP[m]) * w   (scalar_tensor_tensor)
Trig features are fp8 e4m3, built in stage 1: r = S4@[xk;1] (fp32 matmul),
round via +/-C trick (DVE), trig = Sin(2*pi*frac) written straight to fp8.
BN rsqrt via fast-inverse-sqrt bit hack + 2 Newton steps (keeps the ACT
exp table resident; no LoadActFuncSet churn).  The per-block AllGather is
split A(80 queries)/B(16+stats, stats ride as f32 bits in the bf16
payload); chunk A's collective+gather hide under the remaining queries.
BN affines are folded into the MLP/FC weights (per-channel scale on lhsT,
constant shift cancels in the next BN), so no full-width affine sits on
the boundary critical path.  Final FC runs on the gathered o2 directly.

HW-validated constraints: GPSIMD does no compute here (TensorScalarPtr /
PSUM access unsupported); tc.If branches contain only ACT ops (DVE ops
inside If/Else crash the device); DMAs issue on sync/gpsimd queues only;
AluOpType.mod is not valid TensorScalar ISA.
"""
import sys

sys.path.insert(0, "/opt/trn_rl_repo")

import numpy as np
import ml_dtypes

B, N, D, DS, LAT, FD, NF = 2, 384, 128, 3, 256, 1, 2
NB = NF + 1
NQ = 96                  # queries per core
NCH = 24                 # stage-1 chunks (4 queries each)
QA = 80                  # chunk-A query count for the split collective
FREQS = np.linspace(1.0, 32.0, 5).astype(np.float64)
EPS = 1e-5
TWO_PI = float(2 * np.pi)
MAGIC = np.uint32(0x5F3759DF)
C_ROUND = float(3 << 22)  # fp32 round-to-nearest-even trick

BF = ml_dtypes.bfloat16
F8NP = ml_dtypes.float8_e4m3


def _bf(x):
    return np.ascontiguousarray(np.asarray(x, np.float32).astype(BF))


def _f8(x):
    return np.ascontiguousarray(np.asarray(x, np.float32).astype(F8NP))


def _f32(x):
    return np.ascontiguousarray(np.asarray(x, np.float32))


def _wpe_split(Wpe):
    """Ws2 [128,30] trig cols with device sign (-sin trick) folded in:
    row r=10j+t: t<5 -> +Wpe[:,3+6t+j] (sin), t>=5 -> -Wpe[:,3+6(t-5)+3+j]."""
    Ws2 = np.zeros((D, 30), np.float32)
    for j in range(3):
        for t in range(10):
            r = 10 * j + t
            if t < 5:
                Ws2[:, r] = -Wpe[:, 3 + 6 * t + j]
            else:
                Ws2[:, r] = Wpe[:, 3 + 6 * (t - 5) + 3 + j]
    return Ws2, Wpe[:, 0:3].astype(np.float32)


_CACHE = {}


def _build(variant="spmd"):
    if variant in _CACHE:
        return _CACHE[variant]

    import concourse.bacc as bacc
    import concourse.bass as bass
    import concourse.tile as tile
    from concourse import mybir

    F32, BF16 = mybir.dt.float32, mybir.dt.bfloat16
    F8 = mybir.dt.float8e4
    U32 = mybir.dt.uint32
    AF = mybir.ActivationFunctionType
    OP = mybir.AluOpType
    PM = mybir.MatmulPerfMode

    nc = bacc.Bacc(None, target_bir_lowering=False,
                   num_devices=(8 if variant == "spmd" else 1))

    def din(name, shape, dt=BF16):
        return nc.dram_tensor(name, shape, dt, kind="ExternalInput")

    # per-core inputs
    xk_f = din("xk_f", [3, 384], F32)
    s4_f = din("s4_f", [4, NCH * 128], F32)
    xk_b = din("xk_b", [3, 384])
    xq_b = din("xq_b", [3, 96])
    feats_b = din("feats_b", [1, 384])
    feats_q = din("feats_q", [1, 96])
    # replicated inputs
    feats_row = din("feats_row", [1, 768])
    L_Wg1 = din("L_Wg1", [NB, 128, 128])
    L_Wg2 = din("L_Wg2", [NB, 128, 128])
    L_nWk = din("L_nWk", [NB, 128, 128])
    L_Wv = din("L_Wv", [NB, 128, 128])
    L_G1Q = din("L_G1Q", [NB, 128, 128])
    L_G1P8 = din("L_G1P8", [NB, 128, 128], F8)
    L_PsI8 = din("L_PsI8", [NB * 4, 128, 256], F8)
    sp3 = din("sp3", [3, NB * 3 * 128])       # nPd4 | G1Pd4 | Pd4 per block
    sp1 = din("sp1", [1, NB * 2 * 128])       # c1 | bpe per block
    vec_pack = din("vec_pack", [128, 24], F32)
    L_enc = din("L_enc", [1, 128])
    L_em1 = din("L_em1", [NF, 128, 128])
    L_em2 = din("L_em2", [NF, 128, 128])
    L_f1 = din("L_f1", [2, 128, 128])
    L_f2 = din("L_f2", [128, 4 * 128])        # f2[h,k] at col (2h+k)*128

    out_d = nc.dram_tensor("out", [2, 256], F32, kind="ExternalOutput")
    RG = [[0, 1, 2, 3, 4, 5, 6, 7]]

    with tile.TileContext(nc) as tc:
        with (
            tc.tile_pool(name="sing", bufs=1) as sing,
            tc.tile_pool(name="fpool", bufs=2) as fpool,
            tc.tile_pool(name="blk", bufs=2) as blk,
            tc.tile_pool(name="hp", bufs=6) as hp,
            tc.tile_pool(name="wp", bufs=6) as wp,
            tc.tile_pool(name="st1", bufs=2) as st1,
            tc.tile_pool(name="wide", bufs=2) as wide,
            tc.tile_pool(name="smalls", bufs=4) as smalls,
            tc.tile_pool(name="ps_a", bufs=2, space="PSUM") as ps_a,
            tc.tile_pool(name="ps_b", bufs=2, space="PSUM") as ps_b,
            tc.tile_pool(name="ps_g", bufs=4, space="PSUM") as ps_g,
            tc.tile_pool(name="dram", bufs=1, space="DRAM") as dram,
        ):
            def load(src, shape, dt=BF16, pool=sing, tag=None, q=None):
                t = pool.tile(shape, dt, tag=tag, name=tag or "ld")
                (q or nc.sync).dma_start(out=t, in_=src)
                return t

            def loadfam(srcT, nblk, tag, dt=BF16, w=128, q=None):
                t = sing.tile([128, nblk * w], dt, tag=tag, name=tag)
                ap = srcT[:]
                s = bass.AP(tensor=ap.tensor, offset=ap.offset,
                            ap=[[w, 128], [128 * w, nblk], [1, w]])
                (q or nc.sync).dma_start(
                    out=t.rearrange("p (i c) -> p i c", i=nblk), in_=s)
                return [t[:, i * w:(i + 1) * w] for i in range(nblk)]

            # ---- critical loads for stage 1 ----
            xko = sing.tile([4, 384], F32, tag="xko")
            nc.vector.memset(xko, 1.0)
            nc.sync.dma_start(out=xko[0:3, :], in_=xk_f[:])
            s4_sb = load(s4_f[:], [4, NCH * 128], F32, tag="s4")

            vp = load(vec_pack[:], [128, 24], F32, tag="vp")
            magic = vp[:, 22:23]
            negpi = vp[:, 23:24]

            # fp8 mega tile: [VK8 | 24 trig chunks] each 384 cols
            T8 = sing.tile([128, (1 + NCH) * 384], F8, tag="T8", name="T8")
            t8ap = T8[:]

            def t8_slot(c):
                return T8[:, (1 + c) * 384:(2 + c) * 384]

            # ---------- stage 1: trig via S4 matmul + round + Sin -------
            for c in range(NCH):
                rp = ps_a.tile([128, 512], F32, tag="pa")
                nc.tensor.matmul(rp[:, 0:384], s4_sb[:, c * 128:(c + 1) * 128],
                                 xko, start=True, stop=True)
                n_t = st1.tile([128, 384], F32, tag="nt")
                nc.vector.tensor_scalar(out=n_t, in0=rp[:, 0:384],
                                        scalar1=C_ROUND, scalar2=C_ROUND,
                                        op0=OP.add, op1=OP.subtract)
                n_s = st1.tile([128, 384], F32, tag="ns")
                nc.vector.tensor_tensor(out=n_s, in0=rp[:, 0:384], in1=n_t,
                                        op=OP.subtract)
                nc.scalar.activation(out=t8_slot(c), in_=n_s, func=AF.Sin,
                                     bias=0.0, scale=TWO_PI)

            # ---- bulk loads (queue behind stage-1 issues) ----
            W = {}
            W['xkb'] = load(xk_b[:], [3, 384], tag="sxkb")
            W['xqb'] = load(xq_b[:], [3, 96], tag="sxqb")
            W['featsb'] = load(feats_b[:], [1, 384], tag="sfb")
            W['featsq'] = load(feats_q[:], [1, 96], tag="sfq")
            W['feats'] = load(feats_row[:], [1, 768], tag="sfr")
            W['Wg1'] = loadfam(L_Wg1, NB, "wg1")
            W['Wg2'] = loadfam(L_Wg2, NB, "wg2")
            W['nWk'] = loadfam(L_nWk, NB, "nwk")
            W['Wv'] = loadfam(L_Wv, NB, "wv", q=nc.gpsimd)
            W['G1Q'] = loadfam(L_G1Q, NB, "g1q", q=nc.gpsimd)
            W['G1P8'] = loadfam(L_G1P8, NB, "g1p8", dt=F8, q=nc.gpsimd)
            psi = loadfam(L_PsI8, NB * 4, "psi8", dt=F8, w=256, q=nc.gpsimd)
            W['PsI8'] = [[psi[4 * i + s].rearrange("p (s2 k) -> p s2 k", s2=2)
                          for s in range(4)] for i in range(NB)]
            sp3_sb = load(sp3[:], [3, NB * 3 * 128], tag="sp3", q=nc.gpsimd)
            sp1_sb = load(sp1[:], [1, NB * 2 * 128], tag="sp1", q=nc.gpsimd)
            W['enc'] = load(L_enc[:], [1, 128], tag="enc", q=nc.gpsimd)
            W['em1'] = loadfam(L_em1, NF, "em1", q=nc.gpsimd)
            W['em2'] = loadfam(L_em2, NF, "em2", q=nc.gpsimd)
            W['f1'] = loadfam(L_f1, 2, "f1", q=nc.gpsimd)
            f2all = load(L_f2[:], [128, 4 * 128], tag="f2", q=nc.gpsimd)
            W['f2'] = [[f2all[:, (2 * h + k) * 128:(2 * h + k + 1) * 128]
                        for k in range(2)] for h in range(2)]

            def blkslice(base, i, j, w3):
                return base[:, (i * 3 + j) * 128:(i * 3 + j) * 128 + 128]

            W['nPd4'] = [sp3_sb[:, (i * 3 + 0) * 128:(i * 3 + 0) * 128 + 128]
                         for i in range(NB)]
            W['G1Pd4'] = [sp3_sb[:, (i * 3 + 1) * 128:(i * 3 + 1) * 128 + 128]
                          for i in range(NB)]
            W['Pd4'] = [sp3_sb[:, (i * 3 + 2) * 128:(i * 3 + 2) * 128 + 128]
                        for i in range(NB)]
            W['c1'] = [sp1_sb[:, (2 * i) * 128:(2 * i) * 128 + 128]
                       for i in range(NB)]
            W['bpe'] = [sp1_sb[:, (2 * i + 1) * 128:(2 * i + 1) * 128 + 128]
                        for i in range(NB)]
            W['bg2'] = [vp[:, i:i + 1] for i in range(NB)]
            W['gam'] = [vp[:, 3 + i:4 + i] for i in range(NB)]
            W['bet'] = [vp[:, 6 + i:7 + i] for i in range(NB)]
            W['encb'] = vp[:, 9:10]
            W['emb1'] = [vp[:, 10 + j:11 + j] for j in range(NF)]
            W['emb2'] = [vp[:, 12 + j:13 + j] for j in range(NF)]
            W['emg'] = [vp[:, 14 + j:15 + j] for j in range(NF)]
            W['embe'] = [vp[:, 16 + j:17 + j] for j in range(NF)]
            W['f1b'] = [vp[:, 18 + h:19 + h] for h in range(2)]
            W['f2b'] = [vp[:, 20 + h:21 + h] for h in range(2)]
            magic = vp[:, 22:23]
            negpi = vp[:, 23:24]

            ones96 = sing.tile([1, 96], BF16, tag="ones96")
            nc.vector.memset(ones96, 1.0)
            ones384 = sing.tile([128, 384], BF16, tag="ones384")
            nc.vector.memset(ones384, 1.0)
            dumA = sing.tile([128, 1], BF16, tag="dumA")

            # ---------- initial features (ps_b: overlaps stage-1) ----------
            fb = fpool.tile([128, 384], BF16, tag="fb")
            p = ps_b.tile([128, 512], F32, tag="pb")
            nc.tensor.matmul(p[:, 0:384], W['enc'], W['featsb'], start=True,
                             stop=True)
            nc.scalar.activation(out=fb, in_=p[:, 0:384], func=AF.Identity,
                                 bias=W['encb'], scale=1.0)
            fq = fpool.tile([128, 96], BF16, tag="fq")
            p = ps_b.tile([128, 512], F32, tag="pb")
            nc.tensor.matmul(p[:, 0:96], W['enc'], W['featsq'], start=True,
                             stop=True)
            nc.scalar.activation(out=fq, in_=p[:, 0:96], func=AF.Identity,
                                 bias=W['encb'], scale=1.0)

            pid = nc.scalar.partition_id()

            def rsqrt(var_ap, tag):
                """y ~ 1/sqrt(var + EPS): bit hack + 2 Newton steps (DVE)."""
                u = smalls.tile([128, 1], F32, tag="rq_u")
                nc.vector.tensor_scalar(out=u, in0=var_ap, scalar1=EPS,
                                        scalar2=None, op0=OP.add)
                h = smalls.tile([128, 1], U32, tag="rq_h")
                nc.vector.tensor_scalar(out=h, in0=u[:].bitcast(U32),
                                        scalar1=1, scalar2=None,
                                        op0=OP.logical_shift_right)
                y = smalls.tile([128, 1], F32, tag=tag, name=tag)
                nc.vector.tensor_tensor(out=y[:].bitcast(U32),
                                        in0=magic.bitcast(U32), in1=h,
                                        op=OP.subtract)
                for _ in range(2):
                    t1 = smalls.tile([128, 1], F32, tag="rq_t1")
                    nc.vector.tensor_tensor(out=t1, in0=y, in1=y, op=OP.mult)
                    nc.vector.tensor_tensor(out=t1, in0=u, in1=t1, op=OP.mult)
                    nc.vector.tensor_scalar(out=t1, in0=t1, scalar1=-0.5,
                                            scalar2=1.5, op0=OP.mult,
                                            op1=OP.add)
                    y2 = smalls.tile([128, 1], F32, tag=tag, name=tag)
                    nc.vector.tensor_tensor(out=y2, in0=y, in1=t1, op=OP.mult)
                    y = y2
                return y

            # ---------- transformer blocks ----------
            for i in range(NB):
                # block consts: EK(bf16), VK8(fp8), QB, QP  (ps_b ring)
                pa = ps_b.tile([128, 512], F32, tag="pb")
                nc.tensor.matmul(pa[:, 0:384], W['nWk'][i], fb, start=True,
                                 stop=False)
                nc.tensor.matmul(pa[:, 0:384], W['nPd4'][i][0:3, :], W['xkb'],
                                 start=False, stop=True)
                EK = blk.tile([128, 384], BF16, tag="EK")
                nc.scalar.copy(EK, pa[:, 0:384])

                pb = ps_b.tile([128, 512], F32, tag="pb")
                nc.tensor.matmul(pb[:, 0:384], W['Wv'][i], fb, start=True,
                                 stop=False)
                nc.tensor.matmul(pb[:, 0:384], W['nPd4'][i][0:3, :], W['xkb'],
                                 start=False, stop=True)
                nc.scalar.copy(T8[:, 0:384], pb[:, 0:384])   # VK8 fp8

                pa = ps_b.tile([128, 512], F32, tag="pb")
                nc.tensor.matmul(pa[:, 0:96], W['G1Q'][i], fq, start=True,
                                 stop=False)
                nc.tensor.matmul(pa[:, 0:96], W['G1Pd4'][i][0:3, :], W['xqb'],
                                 start=False, stop=False)
                nc.tensor.matmul(pa[:, 0:96], W['c1'][i][0:1, :], ones96,
                                 start=False, stop=True)
                QB = blk.tile([128, 96], F32, tag="QB")
                nc.vector.tensor_copy(QB, pa[:, 0:96])

                pb = ps_b.tile([128, 512], F32, tag="pb")
                nc.tensor.matmul(pb[:, 0:96], W['Pd4'][i][0:3, :], W['xqb'],
                                 start=True, stop=False)
                nc.tensor.matmul(pb[:, 0:96], W['bpe'][i][0:1, :], ones96,
                                 start=False, stop=True)
                QP = blk.tile([128, 96], F32, tag="QP")
                nc.vector.tensor_copy(QP, pb[:, 0:96])

                S1 = blk.tile([128, 96], F32, tag="S1")
                S0 = blk.tile([128, 96], F32, tag="S0")
                # payload: 96 bf16 o-cols + 2 f32 stats (as raw bits)
                P16 = blk.tile([128, 100], BF16, tag="P16")
                P32 = P16[:].bitcast(F32)          # [128, 50]

                ag_in_A = dram.tile([128, QA], BF16, tag=f"aginA{i}")
                ag_in_B = dram.tile([128, 100 - QA], BF16, tag=f"aginB{i}")
                shr = dict(addr_space="Shared") if variant == "spmd" else {}
                ag_out_A = dram.tile([8, 128, QA], BF16, tag=f"agoutA{i}",
                                     **shr)
                ag_out_B = dram.tile([8, 128, 100 - QA], BF16,
                                     tag=f"agoutB{i}", **shr)

                def fan(ag_in, ag_out, src_ap, queues):
                    nc.sync.dma_start(out=ag_in, in_=src_ap)
                    if variant == "spmd":
                        nc.gpsimd.collective_compute(
                            "AllGather", OP.bypass, replica_groups=RG,
                            ins=[ag_in[:].opt()], outs=[ag_out[:].opt()])
                    else:
                        for cc in range(8):
                            queues[cc % len(queues)].dma_start(
                                out=ag_out[cc], in_=src_ap)

                # gathered tile: [p, core, 100] bf16 (o cols 0:96 + stat bits)
                o_ext = wide.tile([128, 8, 100], BF16, tag="oext")
                oe = o_ext[:]
                oe32 = oe.bitcast(F32)             # [p, 8, 50]

                def o_half(h):
                    """[128, 384] bf16 view: cores 4h..4h+4, o cols 0:96."""
                    return o_ext[:, 4 * h:4 * h + 4, 0:96]

                # attention, 1-query software pipeline: relu(m+1) is
                # emitted before stt(m) so DVE never head-of-line blocks;
                # relu 1/3 ACT 2/3 DVE, exp+accum ACT, S1 stt DVE, fp8 DR pos
                def hpre_relu(m):
                    c, s = m // 4, m % 4
                    tsl8 = T8[32 * s:32 * s + 32,
                              (1 + c) * 384:(1 + c) * 384 + 384]
                    pa = ps_a.tile([128, 512], F32, tag="pa")
                    nc.tensor.matmul(pa[:, 0:384],
                                     W['G1P8'][i][32 * s:32 * s + 32, :],
                                     tsl8, start=True, stop=False,
                                     tile_position=(32 * s, 0))
                    nc.tensor.matmul(pa[:, 0:384], W['Wg1'][i], EK,
                                     start=False, stop=True)
                    h_t = hp.tile([128, 384], BF16, tag="h")
                    if m % 3 == 0:
                        nc.scalar.activation(out=h_t, in_=pa[:, 0:384],
                                             func=AF.Relu,
                                             bias=QB[:, m:m + 1], scale=1.0)
                    else:
                        nc.vector.tensor_scalar(
                            out=h_t, in0=pa[:, 0:384],
                            scalar1=QB[:, m:m + 1],
                            scalar2=0.0, op0=OP.add, op1=OP.max)
                    return h_t

                hq = [hpre_relu(0), hpre_relu(1)]
                for m in range(NQ):
                    c, s = m // 4, m % 4
                    lg = ps_g.tile([128, 512], F32, tag="lg")
                    nc.tensor.matmul(lg[:, 0:384], W['Wg2'][i], hq.pop(0),
                                     start=True, stop=True)
                    if m + 2 < NQ:
                        hq.append(hpre_relu(m + 2))
                    w_t = wp.tile([128, 384], BF16, tag="w")
                    nc.scalar.activation(out=w_t, in_=lg[:, 0:384],
                                         func=AF.Exp, bias=W['bg2'][i],
                                         scale=1.0, accum_out=S0[:, m:m + 1])
                    pos = ps_b.tile([128, 512], F32, tag="pb")
                    rhs = bass.AP(tensor=t8ap.tensor, offset=t8ap.offset,
                                  ap=[list(t8ap.ap[0]), [(1 + c) * 384, 2],
                                      [1, 384]])
                    nc.tensor.matmul(pos[:, 0:384], W['PsI8'][i][s], rhs,
                                     start=True, stop=True,
                                     perf_mode=PM.DoubleRow)
                    nc.vector.scalar_tensor_tensor(
                        out=dumA.broadcast_to((128, 384)),
                        in0=pos[:, 0:384], scalar=QP[:, m:m + 1],
                        in1=w_t, op0=OP.add, op1=OP.mult,
                        accum_out=S1[:, m:m + 1])

                    if m == QA + 3:
                        # chunk A payload + collective (hidden under B's
                        # attention)
                        R = smalls.tile([128, QA], F32, tag="RA")
                        nc.vector.reciprocal(out=R, in_=S0[:, 0:QA])
                        nc.vector.tensor_tensor(out=R, in0=S1[:, 0:QA],
                                                in1=R, op=OP.mult)
                        nc.vector.tensor_tensor(out=P16[:, 0:QA], in0=R,
                                                in1=fq[:, 0:QA], op=OP.add)
                        fan(ag_in_A, ag_out_A, P16[:, 0:QA], [nc.sync])
                        src = bass.AP(
                            tensor=ag_out_A[:].tensor,
                            offset=ag_out_A[:].offset,
                            ap=[[QA, 128], [128 * QA, 8], [1, QA]])
                        nc.sync.dma_start(out=o_ext[:, :, 0:QA], in_=src)

                # chunk B payload + stats
                R = smalls.tile([128, 16], F32, tag="RB")
                nc.vector.reciprocal(out=R, in_=S0[:, QA:96])
                nc.vector.tensor_tensor(out=R, in0=S1[:, QA:96], in1=R,
                                        op=OP.mult)
                nc.vector.tensor_tensor(out=P16[:, QA:96], in0=R,
                                        in1=fq[:, QA:96], op=OP.add)
                st = smalls.tile([128, 6], F32, tag="bnst")
                nc.vector.bn_stats(out=st, in_=P16[:, 0:96])
                mv = smalls.tile([128, 2], F32, tag="bnmv")
                nc.vector.bn_aggr(out=mv, in_=st)
                nc.vector.tensor_copy(P32[:, 48:49], mv[:, 0:1])
                msq = smalls.tile([128, 1], F32, tag="msq")
                nc.vector.tensor_tensor(out=msq, in0=mv[:, 0:1],
                                        in1=mv[:, 0:1], op=OP.mult)
                nc.vector.tensor_tensor(out=P32[:, 49:50], in0=mv[:, 1:2],
                                        in1=msq, op=OP.add)
                fan(ag_in_B, ag_out_B, P16[:, QA:100],
                    [nc.sync, nc.gpsimd, nc.sync, nc.gpsimd, nc.sync,
                     nc.gpsimd, nc.sync, nc.sync])
                srcB = bass.AP(tensor=ag_out_B[:].tensor,
                               offset=ag_out_B[:].offset,
                               ap=[[100 - QA, 128], [128 * (100 - QA), 8],
                                   [1, 100 - QA]])
                nc.sync.dma_start(out=o_ext[:, :, QA:100], in_=srcB)

                mg = smalls.tile([128, 1], F32, tag="mg")
                stm = bass.AP(tensor=oe32.tensor, offset=oe32.offset + 48,
                              ap=[list(oe32.ap[0]), [50, 8]])
                nc.vector.tensor_reduce(out=mg, in_=stm,
                                        axis=mybir.AxisListType.X, op=OP.add)
                nc.vector.tensor_scalar(out=mg, in0=mg, scalar1=0.125,
                                        scalar2=None, op0=OP.mult)
                e2g = smalls.tile([128, 1], F32, tag="e2g")
                ste = bass.AP(tensor=oe32.tensor, offset=oe32.offset + 49,
                              ap=[list(oe32.ap[0]), [50, 8]])
                nc.vector.tensor_reduce(out=e2g, in_=ste,
                                        axis=mybir.AxisListType.X, op=OP.add)
                nc.vector.tensor_scalar(out=e2g, in0=e2g, scalar1=0.125,
                                        scalar2=None, op0=OP.mult)
                var = smalls.tile([128, 1], F32, tag="var")
                nc.vector.tensor_tensor(out=var, in0=mg, in1=mg, op=OP.mult)
                nc.vector.tensor_tensor(out=var, in0=e2g, in1=var,
                                        op=OP.subtract)
                rs = rsqrt(var, "rs")
                sc = smalls.tile([128, 1], F32, tag="sc")
                nc.vector.tensor_tensor(out=sc, in0=W['gam'][i], in1=rs,
                                        op=OP.mult)
                b2 = smalls.tile([128, 1], F32, tag="b2")
                nc.vector.tensor_scalar(out=b2, in0=mg, scalar1=sc,
                                        scalar2=None, op0=OP.mult)
                nc.vector.tensor_tensor(out=b2, in0=W['bet'][i], in1=b2,
                                        op=OP.subtract)

                if i < NB - 1:
                    fq = fpool.tile([128, 96], BF16, tag="fq")
                    nc.vector.tensor_scalar(out=fq, in0=P16[:, 0:96],
                                            scalar1=sc, scalar2=b2,
                                            op0=OP.mult, op1=OP.add)
                if i == NB - 1:
                    fb = None                     # last block: no next EK/VK
                else:
                    fb = fpool.tile([128, 384], BF16, tag="fb")
                    with tc.If(pid < 4) as cmp:
                        nc.scalar.activation(out=fb, in_=o_half(0),
                                             func=AF.Identity, bias=b2,
                                             scale=sc)
                    with cmp.Else():
                        nc.scalar.activation(out=fb, in_=o_half(1),
                                             func=AF.Identity, bias=b2,
                                             scale=sc)

                # ---------- MLP with BN1 affine folded into layer 1 ----------
                if i > 0:
                    j = i - 1
                    # W1' = W1 * diag(sc)  (scale lhsT rows);  bias1' =
                    # W1 @ b2 + em_b1.  The +b2 shift of the residual input
                    # cancels inside BN2, so o2' = sc*o + y2 suffices.
                    em1s = wide.tile([128, 128], BF16, tag="em1s")
                    nc.vector.tensor_scalar(out=em1s, in0=W['em1'][j],
                                            scalar1=sc, scalar2=None,
                                            op0=OP.mult)
                    b2b = smalls.tile([128, 1], BF16, tag="b2b")
                    nc.vector.tensor_copy(b2b, b2)
                    pb1 = ps_b.tile([128, 512], F32, tag="pb")
                    nc.tensor.matmul(pb1[:, 0:1], W['em1'][j], b2b,
                                     start=True, stop=True)
                    bias1 = smalls.tile([128, 1], F32, tag="bias1")
                    nc.vector.tensor_tensor(out=bias1, in0=pb1[:, 0:1],
                                            in1=W['emb1'][j], op=OP.add)

                    def mlp_layer(lw, bias_ap, xins, width, tag):
                        t = wide.tile([128, width], BF16, tag=tag)
                        for hh, xin in enumerate(xins):
                            wdt = min(384, width - hh * 384)
                            pp = ps_a.tile([128, 512], F32, tag="pa")
                            nc.tensor.matmul(pp[:, 0:wdt], lw, xin,
                                             start=True, stop=True)
                            if hh % 2 == 0:
                                nc.scalar.activation(
                                    out=t[:, hh * 384:hh * 384 + wdt],
                                    in_=pp[:, 0:wdt], func=AF.Relu,
                                    bias=bias_ap, scale=1.0)
                            else:
                                nc.vector.tensor_scalar(
                                    out=t[:, hh * 384:hh * 384 + wdt],
                                    in0=pp[:, 0:wdt], scalar1=bias_ap,
                                    scalar2=0.0, op0=OP.add, op1=OP.max)
                        return t

                    y1f = mlp_layer(em1s, bias1, [o_half(0), o_half(1)],
                                    768, "y1f")
                    y2f = mlp_layer(W['em2'][j], W['emb2'][j],
                                    [y1f[:, 0:384], y1f[:, 384:768]],
                                    768, "y2f")
                    o2f = wide.tile([128, 768], BF16, tag="o2f")
                    nc.vector.scalar_tensor_tensor(
                        out=o2f[:, 0:384], in0=o_half(0), scalar=sc,
                        in1=y2f[:, 0:384], op0=OP.mult, op1=OP.add)
                    nc.vector.scalar_tensor_tensor(
                        out=o2f[:, 384:768], in0=o_half(1), scalar=sc,
                        in1=y2f[:, 384:768], op0=OP.mult, op1=OP.add)
                    if i < NB - 1:
                        y1q = mlp_layer(em1s, bias1, [P16[:, 0:96]], 96,
                                        "y1q")
                        y2q = mlp_layer(W['em2'][j], W['emb2'][j], [y1q], 96,
                                        "y2q")
                        o2q = wide.tile([128, 96], BF16, tag="o2q")
                        nc.vector.scalar_tensor_tensor(
                            out=o2q, in0=P16[:, 0:96], scalar=sc, in1=y2q,
                            op0=OP.mult, op1=OP.add)

                    st2 = smalls.tile([128, 2, 6], F32, tag="st2")
                    nc.vector.bn_stats(out=st2[:, 0, :], in_=o2f[:, 0:384])
                    nc.vector.bn_stats(out=st2[:, 1, :], in_=o2f[:, 384:768])
                    mv2 = smalls.tile([128, 2], F32, tag="mv2")
                    nc.vector.bn_aggr(out=mv2, in_=st2)
                    rs2 = rsqrt(mv2[:, 1:2], "rs2")
                    sc2 = smalls.tile([128, 1], F32, tag="sc")
                    nc.vector.tensor_tensor(out=sc2, in0=W['emg'][j], in1=rs2,
                                            op=OP.mult)
                    b22 = smalls.tile([128, 1], F32, tag="b2")
                    nc.vector.tensor_scalar(out=b22, in0=mv2[:, 0:1],
                                            scalar1=sc2, scalar2=None,
                                            op0=OP.mult)
                    nc.vector.tensor_tensor(out=b22, in0=W['embe'][j],
                                            in1=b22, op=OP.subtract)
                    if i < NB - 1:
                        fq = fpool.tile([128, 96], BF16, tag="fq")
                        nc.vector.tensor_scalar(out=fq, in0=o2q, scalar1=sc2,
                                                scalar2=b22, op0=OP.mult,
                                                op1=OP.add)
                    if i < NB - 1:
                        fb = fpool.tile([128, 384], BF16, tag="fb")
                        with tc.If(pid < 4) as cmp:
                            nc.scalar.activation(out=fb, in_=o2f[:, 0:384],
                                                 func=AF.Identity, bias=b22,
                                                 scale=sc2)
                        with cmp.Else():
                            nc.scalar.activation(out=fb, in_=o2f[:, 384:768],
                                                 func=AF.Identity, bias=b22,
                                                 scale=sc2)

            # ---------- final FC + max (BN2 affine folded into f1) ----------
            om = smalls.tile([128, 2, 2], F32, tag="om")
            b22b = smalls.tile([128, 1], BF16, tag="b22b")
            nc.vector.tensor_copy(b22b, b22)
            f1s, fc_b1 = [], []
            for h in range(2):
                t = wide.tile([128, 128], BF16, tag=f"f1s{h}")
                nc.vector.tensor_scalar(out=t, in0=W['f1'][h], scalar1=sc2,
                                        scalar2=None, op0=OP.mult)
                f1s.append(t)
                pb1 = ps_b.tile([128, 512], F32, tag="pb")
                nc.tensor.matmul(pb1[:, 0:1], W['f1'][h], b22b,
                                 start=True, stop=True)
                bb1 = smalls.tile([128, 1], F32, tag=f"fcb{h}")
                nc.vector.tensor_tensor(out=bb1, in0=pb1[:, 0:1],
                                        in1=W['f1b'][h], op=OP.add)
                fc_b1.append(bb1)
            for bb in range(2):
                fbb = o2f[:, bb * 384:(bb + 1) * 384]
                e1 = []
                for h in range(2):
                    pp = ps_a.tile([128, 512], F32, tag="pa")
                    nc.tensor.matmul(pp[:, 0:384], f1s[h], fbb,
                                     start=True, stop=True)
                    e1t = wide.tile([128, 384], BF16, tag=f"e1{h}")
                    if h == 0:
                        nc.scalar.activation(out=e1t, in_=pp[:, 0:384],
                                             func=AF.Relu, bias=fc_b1[h],
                                             scale=1.0)
                    else:
                        nc.vector.tensor_scalar(out=e1t, in0=pp[:, 0:384],
                                                scalar1=fc_b1[h],
                                                scalar2=0.0, op0=OP.add,
                                                op1=OP.max)
                    e1.append(e1t)
                for h in range(2):
                    pp = ps_a.tile([128, 512], F32, tag="pa")
                    nc.tensor.matmul(pp[:, 0:384], W['f2'][h][0], e1[0],
                                     start=True, stop=False)
                    nc.tensor.matmul(pp[:, 0:384], W['f2'][h][1], e1[1],
                                     start=False, stop=True)
                    mx = smalls.tile([128, 1], F32, tag="mx")
                    nc.vector.tensor_reduce(out=mx, in_=pp[:, 0:384],
                                            axis=mybir.AxisListType.X,
                                            op=OP.max)
                    nc.vector.tensor_scalar(out=om[:, bb, h:h + 1], in0=mx,
                                            scalar1=W['f2b'][h], scalar2=None,
                                            op0=OP.add)
            dst = bass.AP(tensor=out_d[:].tensor, offset=out_d[:].offset,
                          ap=[[1, 128], [256, 2], [128, 2]])
            nc.sync.dma_start(out=dst, in_=om)

    nc.compile()
    _CACHE[variant] = nc
    return nc


def _prep_inputs(inputs):
    """Host-side constant relayout + per-core slicing. Returns in_maps list."""
    xyz = _f32(inputs["xyz"])          # [2, 384, 3]
    feats = _f32(inputs["feats"])      # [2, 384, 1]

    Wq, Wk, Wv = inputs["tb_Wq"], inputs["tb_Wk"], inputs["tb_Wv"]
    Wg1, bg1 = inputs["tb_Wg1"], inputs["tb_bg1"]
    Wg2, bg2 = inputs["tb_Wg2"], inputs["tb_bg2"]
    Wpe, bpe = inputs["tb_Wpe"], inputs["tb_bpe"]

    L_nWk = np.zeros((NB, 128, 128), np.float32)
    L_Wv = np.zeros((NB, 128, 128), np.float32)
    L_G1Q = np.zeros((NB, 128, 128), np.float32)
    L_Wg1 = np.zeros((NB, 128, 128), np.float32)
    L_Wg2 = np.zeros((NB, 128, 128), np.float32)
    L_G1P8 = np.zeros((NB, 128, 128), np.float32)
    L_PsI8 = np.zeros((NB, 4, 128, 256), np.float32)
    sp3 = np.zeros((3, NB * 3 * 128), np.float32)
    sp1 = np.zeros((1, NB * 2 * 128), np.float32)
    I128 = np.eye(128, dtype=np.float32)
    for i in range(NB):
        Ws2, Wd = _wpe_split(_f32(Wpe[i]))
        g1 = _f32(Wg1[i])
        L_nWk[i] = (-_f32(Wk[i])).T
        L_Wv[i] = _f32(Wv[i]).T
        L_G1Q[i] = (g1 @ _f32(Wq[i])).T
        L_Wg1[i] = g1.T
        L_Wg2[i] = _f32(Wg2[i]).T
        G1P = (g1 @ Ws2).T                      # [30, 128]
        PsT = Ws2.T                             # [30, 128]
        for s in range(4):
            L_G1P8[i, 32 * s:32 * s + 30, :] = G1P
            L_PsI8[i, s, :, 0:128] = I128
            L_PsI8[i, s, 32 * s:32 * s + 30, 128:256] = PsT
        sp3[:, (i * 3 + 0) * 128:(i * 3 + 0) * 128 + 128] = (-4.0 * Wd).T
        sp3[:, (i * 3 + 1) * 128:(i * 3 + 1) * 128 + 128] = (4.0 * (g1 @ Wd)).T
        sp3[:, (i * 3 + 2) * 128:(i * 3 + 2) * 128 + 128] = (4.0 * Wd).T
        sp1[0, (2 * i) * 128:(2 * i) * 128 + 128] = g1 @ _f32(bpe[i]) + _f32(bg1[i])
        sp1[0, (2 * i + 1) * 128:(2 * i + 1) * 128 + 128] = _f32(bpe[i])

    vec_pack = np.zeros((128, 24), np.float32)
    for i in range(NB):
        vec_pack[:, i] = _f32(bg2[i])
        vec_pack[:, 3 + i] = _f32(inputs["tb_gamma"][i])
        vec_pack[:, 6 + i] = _f32(inputs["tb_beta"][i])
    vec_pack[:, 9] = _f32(inputs["enc_b"])
    for j in range(NF):
        vec_pack[:, 10 + j] = _f32(inputs["em_b1"][j])
        vec_pack[:, 12 + j] = _f32(inputs["em_b2"][j])
        vec_pack[:, 14 + j] = _f32(inputs["em_gamma"][j])
        vec_pack[:, 16 + j] = _f32(inputs["em_beta"][j])
    W1f = _f32(inputs["fcf_W1"])
    vec_pack[:, 18] = _f32(inputs["fcf_b1"])[0:128]
    vec_pack[:, 19] = _f32(inputs["fcf_b1"])[128:256]
    vec_pack[:, 20] = _f32(inputs["fcf_b2"])[0:128]
    vec_pack[:, 21] = _f32(inputs["fcf_b2"])[128:256]
    vec_pack[:, 22] = np.full(128, np.frombuffer(
        MAGIC.tobytes(), dtype=np.float32)[0], np.float32)
    vec_pack[:, 23] = -np.pi

    W2 = _f32(inputs["fcf_W2"])
    L_f2 = np.zeros((128, 4 * 128), np.float32)
    for h in range(2):
        for k in range(2):
            L_f2[:, (2 * h + k) * 128:(2 * h + k + 1) * 128] = \
                W2.T[k * 128:(k + 1) * 128, h * 128:(h + 1) * 128]

    com = {
        "feats_row": _bf(feats.reshape(1, 768)),
        "L_Wg1": _bf(L_Wg1), "L_Wg2": _bf(L_Wg2), "L_nWk": _bf(L_nWk),
        "L_Wv": _bf(L_Wv), "L_G1Q": _bf(L_G1Q),
        "L_G1P8": _f8(L_G1P8),
        "L_PsI8": _f8(L_PsI8.reshape(NB * 4, 128, 256)),
        "sp3": _bf(sp3), "sp1": _bf(sp1), "vec_pack": _f32(vec_pack),
        "L_enc": _bf(_f32(inputs["enc_W"])[:, 0:1].T),
        "L_em1": _bf(np.stack([_f32(inputs["em_W1"][j]).T for j in range(NF)])),
        "L_em2": _bf(np.stack([_f32(inputs["em_W2"][j]).T for j in range(NF)])),
        "L_f1": _bf(W1f.T.reshape(128, 2, 128).transpose(1, 0, 2)),
        "L_f2": _bf(L_f2),
    }

    in_maps = []
    for cix in range(8):
        b, qo = cix // 4, (cix % 4) * 96
        xk = xyz[b].T                      # [3, 384]
        S4 = np.zeros((4, NCH, 128), np.float32)
        for c in range(NCH):
            for qr in range(4):
                qg = qo + 4 * c + qr
                for j in range(3):
                    for t in range(10):
                        col = 32 * qr + 10 * j + t
                        s = np.float32(4.0 * FREQS[t % 5] / TWO_PI)
                        off = np.float32(0.25 if t >= 5 else 0.0)
                        S4[j, c, col] = s
                        S4[3, c, col] = (off + 512.0
                                         - s * np.float32(xyz[b, qg, j]))
        m = dict(com)
        m["xk_f"] = _f32(xk)
        m["s4_f"] = _f32(S4.reshape(4, NCH * 128))
        m["xk_b"] = _bf(xk)
        m["xq_b"] = _bf(xk[:, qo:qo + 96])
        m["feats_b"] = _bf(feats[b].reshape(1, 384))
        m["feats_q"] = _bf(feats[b, qo:qo + 96].reshape(1, 96))
        in_maps.append(m)
    return in_maps


def kernel(**inputs):
    from concourse.bass_utils import run_bass_kernel_spmd

    nc = _build()
    in_maps = _prep_inputs(inputs)
    res = run_bass_kernel_spmd(nc, in_maps, list(range(8)))
    return np.asarray(res.results[0]["out"], np.float32)


if __name__ == "__main__":
    print("smoke build only")


# revision 39
# speedup vs baseline: 1.0023x; 1.0023x over previous
"""NePuEncoder Bass/Tile kernel for 8 Trainium2 NeuronCores.

Sharding: query-parallel. Core c handles batch b=c//4, queries qo=(c%4)*96 ..
qo+96 of that batch. Channel-major layout [128 chan, keys] throughout.

Per-query attention fully fused in SBUF/PSUM:
  - PE:   hpre = G1P8(fp8,K=32)@trig8 + Wg1(bf16)@EK;  logits = Wg2@h;
          pos  = ONE fp8 DoubleRow matmul (I8 (x) VK8  +  PsShift8 (x) trig8)
          (per-query PE cost 560ns vs 800ns all-bf16)
  - relu(hpre+QB[m]): 1/3 on ACT, 2/3 on DVE (balance)
  - ACT:  w = exp(logits + bg2) with accum -> S0
  - DVE:  S1[m] = sum_n (pos + QP[m]) * w   (scalar_tensor_tensor)
Trig features are fp8 e4m3, built in stage 1: r = S4@[xk;1] (fp32 matmul),
round via +/-C trick (DVE), trig = Sin(2*pi*frac) written straight to fp8.
BN rsqrt via fast-inverse-sqrt bit hack + 2 Newton steps (keeps the ACT
exp table resident; no LoadActFuncSet churn).  The per-block AllGather is
split A(80 queries)/B(16+stats, stats ride as f32 bits in the bf16
payload); chunk A's collective+gather hide under the remaining queries.
BN affines are folded into the MLP/FC weights (per-channel scale on lhsT,
constant shift cancels in the next BN), so no full-width affine sits on
the boundary critical path.  Final FC runs on the gathered o2 directly.

HW-validated constraints: GPSIMD does no compute here (TensorScalarPtr /
PSUM access unsupported); tc.If branches contain only ACT ops (DVE ops
inside If/Else crash the device); DMAs issue on sync/gpsimd queues only;
AluOpType.mod is not valid TensorScalar ISA.
"""
import sys

sys.path.insert(0, "/opt/trn_rl_repo")

import numpy as np
import ml_dtypes

B, N, D, DS, LAT, FD, NF = 2, 384, 128, 3, 256, 1, 2
NB = NF + 1
NQ = 96                  # queries per core
NCH = 24                 # stage-1 chunks (4 queries each)
QA = 80                  # chunk-A query count for the split collective
FREQS = np.linspace(1.0, 32.0, 5).astype(np.float64)
EPS = 1e-5
TWO_PI = float(2 * np.pi)
MAGIC = np.uint32(0x5F3759DF)
C_ROUND = float(3 << 22)  # fp32 round-to-nearest-even trick

BF = ml_dtypes.bfloat16
F8NP = ml_dtypes.float8_e4m3


def _bf(x):
    return np.ascontiguousarray(np.asarray(x, np.float32).astype(BF))


def _f8(x):
    return np.ascontiguousarray(np.asarray(x, np.float32).astype(F8NP))


def _f32(x):
    return np.ascontiguousarray(np.asarray(x, np.float32))


def _wpe_split(Wpe):
    """Ws2 [128,30] trig cols with device sign (-sin trick) folded in:
    row r=10j+t: t<5 -> +Wpe[:,3+6t+j] (sin), t>=5 -> -Wpe[:,3+6(t-5)+3+j]."""
    Ws2 = np.zeros((D, 30), np.float32)
    for j in range(3):
        for t in range(10):
            r = 10 * j + t
            if t < 5:
                Ws2[:, r] = -Wpe[:, 3 + 6 * t + j]
            else:
                Ws2[:, r] = Wpe[:, 3 + 6 * (t - 5) + 3 + j]
    return Ws2, Wpe[:, 0:3].astype(np.float32)


_CACHE = {}


def _build(variant="spmd"):
    if variant in _CACHE:
        return _CACHE[variant]

    import concourse.bacc as bacc
    import concourse.bass as bass
    import concourse.tile as tile
    from concourse import mybir

    F32, BF16 = mybir.dt.float32, mybir.dt.bfloat16
    F8 = mybir.dt.float8e4
    U32 = mybir.dt.uint32
    AF = mybir.ActivationFunctionType
    OP = mybir.AluOpType
    PM = mybir.MatmulPerfMode

    nc = bacc.Bacc(None, target_bir_lowering=False,
                   num_devices=(8 if variant == "spmd" else 1))

    def din(name, shape, dt=BF16):
        return nc.dram_tensor(name, shape, dt, kind="ExternalInput")

    # per-core inputs
    xk_f = din("xk_f", [3, 384], F32)
    s4_f = din("s4_f", [4, NCH * 128], F32)
    xk_b = din("xk_b", [3, 384])
    xq_b = din("xq_b", [3, 96])
    feats_b = din("feats_b", [1, 384])
    feats_q = din("feats_q", [1, 96])
    # replicated inputs
    feats_row = din("feats_row", [1, 768])
    L_Wg1 = din("L_Wg1", [NB, 128, 128])
    L_Wg2 = din("L_Wg2", [NB, 128, 128])
    L_nWk = din("L_nWk", [NB, 128, 128])
    L_Wv = din("L_Wv", [NB, 128, 128])
    L_G1Q = din("L_G1Q", [NB, 128, 128])
    L_G1P8 = din("L_G1P8", [NB, 128, 128], F8)
    L_PsI8 = din("L_PsI8", [NB * 4, 128, 256], F8)
    sp3 = din("sp3", [3, NB * 3 * 128])       # nPd4 | G1Pd4 | Pd4 per block
    sp1 = din("sp1", [1, NB * 2 * 128])       # c1 | bpe per block
    vec_pack = din("vec_pack", [128, 24], F32)
    L_enc = din("L_enc", [1, 128])
    L_em1 = din("L_em1", [NF, 128, 128])
    L_em2 = din("L_em2", [NF, 128, 128])
    L_f1 = din("L_f1", [2, 128, 128])
    L_f2 = din("L_f2", [128, 4 * 128])        # f2[h,k] at col (2h+k)*128

    out_d = nc.dram_tensor("out", [2, 256], F32, kind="ExternalOutput")
    RG = [[0, 1, 2, 3, 4, 5, 6, 7]]

    with tile.TileContext(nc) as tc:
        with (
            tc.tile_pool(name="sing", bufs=1) as sing,
            tc.tile_pool(name="fpool", bufs=2) as fpool,
            tc.tile_pool(name="blk", bufs=2) as blk,
            tc.tile_pool(name="hp", bufs=6) as hp,
            tc.tile_pool(name="wp", bufs=6) as wp,
            tc.tile_pool(name="st1", bufs=2) as st1,
            tc.tile_pool(name="wide", bufs=2) as wide,
            tc.tile_pool(name="smalls", bufs=4) as smalls,
            tc.tile_pool(name="ps_a", bufs=2, space="PSUM") as ps_a,
            tc.tile_pool(name="ps_b", bufs=2, space="PSUM") as ps_b,
            tc.tile_pool(name="ps_g", bufs=4, space="PSUM") as ps_g,
            tc.tile_pool(name="dram", bufs=1, space="DRAM") as dram,
        ):
            def load(src, shape, dt=BF16, pool=sing, tag=None, q=None):
                t = pool.tile(shape, dt, tag=tag, name=tag or "ld")
                (q or nc.sync).dma_start(out=t, in_=src)
                return t

            def loadfam(srcT, nblk, tag, dt=BF16, w=128, q=None):
                t = sing.tile([128, nblk * w], dt, tag=tag, name=tag)
                ap = srcT[:]
                s = bass.AP(tensor=ap.tensor, offset=ap.offset,
                            ap=[[w, 128], [128 * w, nblk], [1, w]])
                (q or nc.sync).dma_start(
                    out=t.rearrange("p (i c) -> p i c", i=nblk), in_=s)
                return [t[:, i * w:(i + 1) * w] for i in range(nblk)]

            # ---- critical loads for stage 1 ----
            xko = sing.tile([4, 384], F32, tag="xko")
            nc.vector.memset(xko, 1.0)
            nc.sync.dma_start(out=xko[0:3, :], in_=xk_f[:])
            s4_sb = load(s4_f[:], [4, NCH * 128], F32, tag="s4")

            vp = load(vec_pack[:], [128, 24], F32, tag="vp")
            magic = vp[:, 22:23]
            negpi = vp[:, 23:24]

            # fp8 mega tile: [VK8 | 24 trig chunks] each 384 cols
            T8 = sing.tile([128, (1 + NCH) * 384], F8, tag="T8", name="T8")
            t8ap = T8[:]

            def t8_slot(c):
                return T8[:, (1 + c) * 384:(2 + c) * 384]

            # ---------- stage 1: trig via S4 matmul + round + Sin -------
            for c in range(NCH):
                rp = ps_a.tile([128, 512], F32, tag="pa")
                nc.tensor.matmul(rp[:, 0:384], s4_sb[:, c * 128:(c + 1) * 128],
                                 xko, start=True, stop=True)
                n_t = st1.tile([128, 384], F32, tag="nt")
                nc.vector.tensor_scalar(out=n_t, in0=rp[:, 0:384],
                                        scalar1=C_ROUND, scalar2=C_ROUND,
                                        op0=OP.add, op1=OP.subtract)
                n_s = st1.tile([128, 384], F32, tag="ns")
                nc.vector.tensor_tensor(out=n_s, in0=rp[:, 0:384], in1=n_t,
                                        op=OP.subtract)
                nc.scalar.activation(out=t8_slot(c), in_=n_s, func=AF.Sin,
                                     bias=0.0, scale=TWO_PI)

            # ---- bulk loads (queue behind stage-1 issues) ----
            W = {}
            W['xkb'] = load(xk_b[:], [3, 384], tag="sxkb")
            W['xqb'] = load(xq_b[:], [3, 96], tag="sxqb")
            W['featsb'] = load(feats_b[:], [1, 384], tag="sfb")
            W['featsq'] = load(feats_q[:], [1, 96], tag="sfq")
            W['feats'] = load(feats_row[:], [1, 768], tag="sfr")
            W['Wg1'] = loadfam(L_Wg1, NB, "wg1")
            W['Wg2'] = loadfam(L_Wg2, NB, "wg2")
            W['nWk'] = loadfam(L_nWk, NB, "nwk")
            W['Wv'] = loadfam(L_Wv, NB, "wv", q=nc.gpsimd)
            W['G1Q'] = loadfam(L_G1Q, NB, "g1q", q=nc.gpsimd)
            W['G1P8'] = loadfam(L_G1P8, NB, "g1p8", dt=F8, q=nc.gpsimd)
            psi = loadfam(L_PsI8, NB * 4, "psi8", dt=F8, w=256, q=nc.gpsimd)
            W['PsI8'] = [[psi[4 * i + s].rearrange("p (s2 k) -> p s2 k", s2=2)
                          for s in range(4)] for i in range(NB)]
            sp3_sb = load(sp3[:], [3, NB * 3 * 128], tag="sp3", q=nc.gpsimd)
            sp1_sb = load(sp1[:], [1, NB * 2 * 128], tag="sp1", q=nc.gpsimd)
            W['enc'] = load(L_enc[:], [1, 128], tag="enc", q=nc.gpsimd)
            W['em1'] = loadfam(L_em1, NF, "em1", q=nc.gpsimd)
            W['em2'] = loadfam(L_em2, NF, "em2", q=nc.gpsimd)
            W['f1'] = loadfam(L_f1, 2, "f1", q=nc.gpsimd)
            f2all = load(L_f2[:], [128, 4 * 128], tag="f2", q=nc.gpsimd)
            W['f2'] = [[f2all[:, (2 * h + k) * 128:(2 * h + k + 1) * 128]
                        for k in range(2)] for h in range(2)]

            def blkslice(base, i, j, w3):
                return base[:, (i * 3 + j) * 128:(i * 3 + j) * 128 + 128]

            W['nPd4'] = [sp3_sb[:, (i * 3 + 0) * 128:(i * 3 + 0) * 128 + 128]
                         for i in range(NB)]
            W['G1Pd4'] = [sp3_sb[:, (i * 3 + 1) * 128:(i * 3 + 1) * 128 + 128]
                          for i in range(NB)]
            W['Pd4'] = [sp3_sb[:, (i * 3 + 2) * 128:(i * 3 + 2) * 128 + 128]
                        for i in range(NB)]
            W['c1'] = [sp1_sb[:, (2 * i) * 128:(2 * i) * 128 + 128]
                       for i in range(NB)]
            W['bpe'] = [sp1_sb[:, (2 * i + 1) * 128:(2 * i + 1) * 128 + 128]
                        for i in range(NB)]
            W['bg2'] = [vp[:, i:i + 1] for i in range(NB)]
            W['gam'] = [vp[:, 3 + i:4 + i] for i in range(NB)]
            W['bet'] = [vp[:, 6 + i:7 + i] for i in range(NB)]
            W['encb'] = vp[:, 9:10]
            W['emb1'] = [vp[:, 10 + j:11 + j] for j in range(NF)]
            W['emb2'] = [vp[:, 12 + j:13 + j] for j in range(NF)]
            W['emg'] = [vp[:, 14 + j:15 + j] for j in range(NF)]
            W['embe'] = [vp[:, 16 + j:17 + j] for j in range(NF)]
            W['f1b'] = [vp[:, 18 + h:19 + h] for h in range(2)]
            W['f2b'] = [vp[:, 20 + h:21 + h] for h in range(2)]
            magic = vp[:, 22:23]
            negpi = vp[:, 23:24]

            ones96 = sing.tile([1, 96], BF16, tag="ones96")
            nc.vector.memset(ones96, 1.0)
            ones384 = sing.tile([128, 384], BF16, tag="ones384")
            nc.vector.memset(ones384, 1.0)
            dumA = sing.tile([128, 1], BF16, tag="dumA")

            # ---------- initial features (ps_b: overlaps stage-1) ----------
            fb = fpool.tile([128, 384], BF16, tag="fb")
            p = ps_b.tile([128, 512], F32, tag="pb")
            nc.tensor.matmul(p[:, 0:384], W['enc'], W['featsb'], start=True,
                             stop=True)
            nc.scalar.activation(out=fb, in_=p[:, 0:384], func=AF.Identity,
                                 bias=W['encb'], scale=1.0)
            fq = fpool.tile([128, 96], BF16, tag="fq")
            p = ps_b.tile([128, 512], F32, tag="pb")
            nc.tensor.matmul(p[:, 0:96], W['enc'], W['featsq'], start=True,
                             stop=True)
            nc.scalar.activation(out=fq, in_=p[:, 0:96], func=AF.Identity,
                                 bias=W['encb'], scale=1.0)

            pid = nc.scalar.partition_id()

            def rsqrt(var_ap, tag):
                """y ~ 1/sqrt(var + EPS): bit hack + 2 Newton steps (DVE)."""
                u = smalls.tile([128, 1], F32, tag="rq_u")
                nc.vector.tensor_scalar(out=u, in0=var_ap, scalar1=EPS,
                                        scalar2=None, op0=OP.add)
                h = smalls.tile([128, 1], U32, tag="rq_h")
                nc.vector.tensor_scalar(out=h, in0=u[:].bitcast(U32),
                                        scalar1=1, scalar2=None,
                                        op0=OP.logical_shift_right)
                y = smalls.tile([128, 1], F32, tag=tag, name=tag)
                nc.vector.tensor_tensor(out=y[:].bitcast(U32),
                                        in0=magic.bitcast(U32), in1=h,
                                        op=OP.subtract)
                for _ in range(2):
                    t1 = smalls.tile([128, 1], F32, tag="rq_t1")
                    nc.vector.tensor_tensor(out=t1, in0=y, in1=y, op=OP.mult)
                    nc.vector.tensor_tensor(out=t1, in0=u, in1=t1, op=OP.mult)
                    nc.vector.tensor_scalar(out=t1, in0=t1, scalar1=-0.5,
                                            scalar2=1.5, op0=OP.mult,
                                            op1=OP.add)
                    y2 = smalls.tile([128, 1], F32, tag=tag, name=tag)
                    nc.vector.tensor_tensor(out=y2, in0=y, in1=t1, op=OP.mult)
                    y = y2
                return y

            # ---------- transformer blocks ----------
            for i in range(NB):
                # block consts: EK(bf16), VK8(fp8), QB, QP  (ps_b ring)
                pa = ps_b.tile([128, 512], F32, tag="pb")
                nc.tensor.matmul(pa[:, 0:384], W['nWk'][i], fb, start=True,
                                 stop=False)
                nc.tensor.matmul(pa[:, 0:384], W['nPd4'][i][0:3, :], W['xkb'],
                                 start=False, stop=True)
                EK = blk.tile([128, 384], BF16, tag="EK")
                nc.scalar.copy(EK, pa[:, 0:384])

                pb = ps_b.tile([128, 512], F32, tag="pb")
                nc.tensor.matmul(pb[:, 0:384], W['Wv'][i], fb, start=True,
                                 stop=False)
                nc.tensor.matmul(pb[:, 0:384], W['nPd4'][i][0:3, :], W['xkb'],
                                 start=False, stop=True)
                nc.scalar.copy(T8[:, 0:384], pb[:, 0:384])   # VK8 fp8

                pa = ps_b.tile([128, 512], F32, tag="pb")
                nc.tensor.matmul(pa[:, 0:96], W['G1Q'][i], fq, start=True,
                                 stop=False)
                nc.tensor.matmul(pa[:, 0:96], W['G1Pd4'][i][0:3, :], W['xqb'],
                                 start=False, stop=False)
                nc.tensor.matmul(pa[:, 0:96], W['c1'][i][0:1, :], ones96,
                                 start=False, stop=True)
                QB = blk.tile([128, 96], F32, tag="QB")
                nc.vector.tensor_copy(QB, pa[:, 0:96])

                pb = ps_b.tile([128, 512], F32, tag="pb")
                nc.tensor.matmul(pb[:, 0:96], W['Pd4'][i][0:3, :], W['xqb'],
                                 start=True, stop=False)
                nc.tensor.matmul(pb[:, 0:96], W['bpe'][i][0:1, :], ones96,
                                 start=False, stop=True)
                QP = blk.tile([128, 96], F32, tag="QP")
                nc.vector.tensor_copy(QP, pb[:, 0:96])

                S1 = blk.tile([128, 96], F32, tag="S1")
                S0 = blk.tile([128, 96], F32, tag="S0")
                # payload: 96 bf16 o-cols + 2 f32 stats (as raw bits)
                P16 = blk.tile([128, 100], BF16, tag="P16")
                P32 = P16[:].bitcast(F32)          # [128, 50]

                ag_in_A = dram.tile([128, QA], BF16, tag=f"aginA{i}")
                ag_in_B = dram.tile([128, 100 - QA], BF16, tag=f"aginB{i}")
                shr = dict(addr_space="Shared") if variant == "spmd" else {}
                ag_out_A = dram.tile([8, 128, QA], BF16, tag=f"agoutA{i}",
                                     **shr)
                ag_out_B = dram.tile([8, 128, 100 - QA], BF16,
                                     tag=f"agoutB{i}", **shr)

                def fan(ag_in, ag_out, src_ap, queues):
                    nc.sync.dma_start(out=ag_in, in_=src_ap)
                    if variant == "spmd":
                        nc.gpsimd.collective_compute(
                            "AllGather", OP.bypass, replica_groups=RG,
                            ins=[ag_in[:].opt()], outs=[ag_out[:].opt()])
                    else:
                        for cc in range(8):
                            queues[cc % len(queues)].dma_start(
                                out=ag_out[cc], in_=src_ap)

                # gathered tile: [p, core, 100] bf16 (o cols 0:96 + stat bits)
                o_ext = wide.tile([128, 8, 100], BF16, tag="oext")
                oe = o_ext[:]
                oe32 = oe.bitcast(F32)             # [p, 8, 50]

                def o_half(h):
                    """[128, 384] bf16 view: cores 4h..4h+4, o cols 0:96."""
                    return o_ext[:, 4 * h:4 * h + 4, 0:96]

                # attention, 1-query software pipeline: relu(m+1) is
                # emitted before stt(m) so DVE never head-of-line blocks;
                # relu 1/3 ACT 2/3 DVE, exp+accum ACT, S1 stt DVE, fp8 DR pos
                def hpre_relu(m):
                    c, s = m // 4, m % 4
                    tsl8 = T8[32 * s:32 * s + 32,
                              (1 + c) * 384:(1 + c) * 384 + 384]
                    pa = ps_a.tile([128, 512], F32, tag="pa")
                    nc.tensor.matmul(pa[:, 0:384],
                                     W['G1P8'][i][32 * s:32 * s + 32, :],
                                     tsl8, start=True, stop=False,
                                     tile_position=(32 * s, 0))
                    nc.tensor.matmul(pa[:, 0:384], W['Wg1'][i], EK,
                                     start=False, stop=True)
                    h_t = hp.tile([128, 384], BF16, tag="h")
                    if m % 3 == 0:
                        nc.scalar.activation(out=h_t, in_=pa[:, 0:384],
                                             func=AF.Relu,
                                             bias=QB[:, m:m + 1], scale=1.0)
                    else:
                        nc.vector.tensor_scalar(
                            out=h_t, in0=pa[:, 0:384],
                            scalar1=QB[:, m:m + 1],
                            scalar2=0.0, op0=OP.add, op1=OP.max)
                    return h_t

                hq = [hpre_relu(0), hpre_relu(1)]
                for m in range(NQ):
                    c, s = m // 4, m % 4
                    lg = ps_g.tile([128, 512], F32, tag="lg")
                    nc.tensor.matmul(lg[:, 0:384], W['Wg2'][i], hq.pop(0),
                                     start=True, stop=True)
                    if m + 2 < NQ:
                        hq.append(hpre_relu(m + 2))
                    w_t = wp.tile([128, 384], BF16, tag="w")
                    nc.scalar.activation(out=w_t, in_=lg[:, 0:384],
                                         func=AF.Exp, bias=W['bg2'][i],
                                         scale=1.0, accum_out=S0[:, m:m + 1])
                    pos = ps_b.tile([128, 512], F32, tag="pb")
                    rhs = bass.AP(tensor=t8ap.tensor, offset=t8ap.offset,
                                  ap=[list(t8ap.ap[0]), [(1 + c) * 384, 2],
                                      [1, 384]])
                    nc.tensor.matmul(pos[:, 0:384], W['PsI8'][i][s], rhs,
                                     start=True, stop=True,
                                     perf_mode=PM.DoubleRow)
                    nc.vector.scalar_tensor_tensor(
                        out=dumA.broadcast_to((128, 384)),
                        in0=pos[:, 0:384], scalar=QP[:, m:m + 1],
                        in1=w_t, op0=OP.add, op1=OP.mult,
                        accum_out=S1[:, m:m + 1])

                    if m == QA + 3:
                        # chunk A payload + collective (hidden under B's
                        # attention)
                        R = smalls.tile([128, QA], F32, tag="RA")
                        nc.vector.reciprocal(out=R, in_=S0[:, 0:QA])
                        nc.vector.tensor_tensor(out=R, in0=S1[:, 0:QA],
                                                in1=R, op=OP.mult)
                        nc.vector.tensor_tensor(out=P16[:, 0:QA], in0=R,
                                                in1=fq[:, 0:QA], op=OP.add)
                        fan(ag_in_A, ag_out_A, P16[:, 0:QA], [nc.sync])
                        src = bass.AP(
                            tensor=ag_out_A[:].tensor,
                            offset=ag_out_A[:].offset,
                            ap=[[QA, 128], [128 * QA, 8], [1, QA]])
                        nc.sync.dma_start(out=o_ext[:, :, 0:QA], in_=src)

                # chunk B payload + stats
                R = smalls.tile([128, 16], F32, tag="RB")
                nc.vector.reciprocal(out=R, in_=S0[:, QA:96])
                nc.vector.tensor_tensor(out=R, in0=S1[:, QA:96], in1=R,
                                        op=OP.mult)
                nc.vector.tensor_tensor(out=P16[:, QA:96], in0=R,
                                        in1=fq[:, QA:96], op=OP.add)
                st = smalls.tile([128, 6], F32, tag="bnst")
                nc.vector.bn_stats(out=st, in_=P16[:, 0:96])
                mv = smalls.tile([128, 2], F32, tag="bnmv")
                nc.vector.bn_aggr(out=mv, in_=st)
                nc.vector.tensor_copy(P32[:, 48:49], mv[:, 0:1])
                msq = smalls.tile([128, 1], F32, tag="msq")
                nc.vector.tensor_tensor(out=msq, in0=mv[:, 0:1],
                                        in1=mv[:, 0:1], op=OP.mult)
                nc.vector.tensor_tensor(out=P32[:, 49:50], in0=mv[:, 1:2],
                                        in1=msq, op=OP.add)
                fan(ag_in_B, ag_out_B, P16[:, QA:100],
                    [nc.sync, nc.gpsimd, nc.sync, nc.gpsimd, nc.sync,
                     nc.gpsimd, nc.sync, nc.sync])
                srcB = bass.AP(tensor=ag_out_B[:].tensor,
                               offset=ag_out_B[:].offset,
                               ap=[[100 - QA, 128], [128 * (100 - QA), 8],
                                   [1, 100 - QA]])
                nc.sync.dma_start(out=o_ext[:, :, QA:100], in_=srcB)

                mg = smalls.tile([128, 1], F32, tag="mg")
                stm = bass.AP(tensor=oe32.tensor, offset=oe32.offset + 48,
                              ap=[list(oe32.ap[0]), [50, 8]])
                nc.vector.tensor_reduce(out=mg, in_=stm,
                                        axis=mybir.AxisListType.X, op=OP.add)
                nc.vector.tensor_scalar(out=mg, in0=mg, scalar1=0.125,
                                        scalar2=None, op0=OP.mult)
                e2g = smalls.tile([128, 1], F32, tag="e2g")
                ste = bass.AP(tensor=oe32.tensor, offset=oe32.offset + 49,
                              ap=[list(oe32.ap[0]), [50, 8]])
                nc.vector.tensor_reduce(out=e2g, in_=ste,
                                        axis=mybir.AxisListType.X, op=OP.add)
                nc.vector.tensor_scalar(out=e2g, in0=e2g, scalar1=0.125,
                                        scalar2=None, op0=OP.mult)
                var = smalls.tile([128, 1], F32, tag="var")
                nc.vector.tensor_tensor(out=var, in0=mg, in1=mg, op=OP.mult)
                nc.vector.tensor_tensor(out=var, in0=e2g, in1=var,
                                        op=OP.subtract)
                rs = rsqrt(var, "rs")
                sc = smalls.tile([128, 1], F32, tag="sc")
                nc.vector.tensor_tensor(out=sc, in0=W['gam'][i], in1=rs,
                                        op=OP.mult)
                b2 = smalls.tile([128, 1], F32, tag="b2")
                nc.vector.tensor_scalar(out=b2, in0=mg, scalar1=sc,
                                        scalar2=None, op0=OP.mult)
                nc.vector.tensor_tensor(out=b2, in0=W['bet'][i], in1=b2,
                                        op=OP.subtract)

                if i < NB - 1:
                    fq = fpool.tile([128, 96], BF16, tag="fq")
                    nc.vector.tensor_scalar(out=fq, in0=P16[:, 0:96],
                                            scalar1=sc, scalar2=b2,
                                            op0=OP.mult, op1=OP.add)
                if i == NB - 1:
                    fb = None                     # last block: no next EK/VK
                else:
                    fb = fpool.tile([128, 384], BF16, tag="fb")
                    with tc.If(pid < 4) as cmp:
                        nc.scalar.activation(out=fb, in_=o_half(0),
                                             func=AF.Identity, bias=b2,
                                             scale=sc)
                    with cmp.Else():
                        nc.scalar.activation(out=fb, in_=o_half(1),
                                             func=AF.Identity, bias=b2,
                                             scale=sc)

                # ---------- MLP with BN1 affine folded into layer 1 ----------
                if i > 0:
                    j = i - 1
                    # W1' = W1 * diag(sc)  (scale lhsT rows);  bias1' =
                    # W1 @ b2 + em_b1.  The +b2 shift of the residual input
                    # cancels inside BN2, so o2' = sc*o + y2 suffices.
                    em1s = wide.tile([128, 128], BF16, tag="em1s")
                    nc.vector.tensor_scalar(out=em1s, in0=W['em1'][j],
                                            scalar1=sc, scalar2=None,
                                            op0=OP.mult)
                    b2b = smalls.tile([128, 1], BF16, tag="b2b")
                    nc.vector.tensor_copy(b2b, b2)
                    pb1 = ps_b.tile([128, 512], F32, tag="pb")
                    nc.tensor.matmul(pb1[:, 0:1], W['em1'][j], b2b,
                                     start=True, stop=True)
                    bias1 = smalls.tile([128, 1], F32, tag="bias1")
                    nc.vector.tensor_tensor(out=bias1, in0=pb1[:, 0:1],
                                            in1=W['emb1'][j], op=OP.add)

                    def mlp_layer(lw, bias_ap, xins, width, tag):
                        t = wide.tile([128, width], BF16, tag=tag)
                        for hh, xin in enumerate(xins):
                            wdt = min(384, width - hh * 384)
                            pp = ps_a.tile([128, 512], F32, tag="pa")
                            nc.tensor.matmul(pp[:, 0:wdt], lw, xin,
                                             start=True, stop=True)
                            nc.scalar.activation(
                                out=t[:, hh * 384:hh * 384 + wdt],
                                in_=pp[:, 0:wdt], func=AF.Relu,
                                bias=bias_ap, scale=1.0)
                        return t

                    y1f = mlp_layer(em1s, bias1, [o_half(0), o_half(1)],
                                    768, "y1f")
                    y2f = mlp_layer(W['em2'][j], W['emb2'][j],
                                    [y1f[:, 0:384], y1f[:, 384:768]],
                                    768, "y2f")
                    o2f = wide.tile([128, 768], BF16, tag="o2f")
                    nc.vector.scalar_tensor_tensor(
                        out=o2f[:, 0:384], in0=o_half(0), scalar=sc,
                        in1=y2f[:, 0:384], op0=OP.mult, op1=OP.add)
                    nc.vector.scalar_tensor_tensor(
                        out=o2f[:, 384:768], in0=o_half(1), scalar=sc,
                        in1=y2f[:, 384:768], op0=OP.mult, op1=OP.add)
                    if i < NB - 1:
                        y1q = mlp_layer(em1s, bias1, [P16[:, 0:96]], 96,
                                        "y1q")
                        y2q = mlp_layer(W['em2'][j], W['emb2'][j], [y1q], 96,
                                        "y2q")
                        o2q = wide.tile([128, 96], BF16, tag="o2q")
                        nc.vector.scalar_tensor_tensor(
                            out=o2q, in0=P16[:, 0:96], scalar=sc, in1=y2q,
                            op0=OP.mult, op1=OP.add)

                    st2 = smalls.tile([128, 2, 6], F32, tag="st2")
                    nc.vector.bn_stats(out=st2[:, 0, :], in_=o2f[:, 0:384])
                    nc.vector.bn_stats(out=st2[:, 1, :], in_=o2f[:, 384:768])
                    mv2 = smalls.tile([128, 2], F32, tag="mv2")
                    nc.vector.bn_aggr(out=mv2, in_=st2)
                    rs2 = rsqrt(mv2[:, 1:2], "rs2")
                    sc2 = smalls.tile([128, 1], F32, tag="sc")
                    nc.vector.tensor_tensor(out=sc2, in0=W['emg'][j], in1=rs2,
                                            op=OP.mult)
                    b22 = smalls.tile([128, 1], F32, tag="b2")
                    nc.vector.tensor_scalar(out=b22, in0=mv2[:, 0:1],
                                            scalar1=sc2, scalar2=None,
                                            op0=OP.mult)
                    nc.vector.tensor_tensor(out=b22, in0=W['embe'][j],
                                            in1=b22, op=OP.subtract)
                    if i < NB - 1:
                        fq = fpool.tile([128, 96], BF16, tag="fq")
                        nc.vector.tensor_scalar(out=fq, in0=o2q, scalar1=sc2,
                                                scalar2=b22, op0=OP.mult,
                                                op1=OP.add)
                    if i < NB - 1:
                        fb = fpool.tile([128, 384], BF16, tag="fb")
                        with tc.If(pid < 4) as cmp:
                            nc.scalar.activation(out=fb, in_=o2f[:, 0:384],
                                                 func=AF.Identity, bias=b22,
                                                 scale=sc2)
                        with cmp.Else():
                            nc.scalar.activation(out=fb, in_=o2f[:, 384:768],
                                                 func=AF.Identity, bias=b22,
                                                 scale=sc2)

            # ---------- final FC + max (BN2 affine folded into f1) ----------
            om = smalls.tile([128, 2, 2], F32, tag="om")
            b22b = smalls.tile([128, 1], BF16, tag="b22b")
            nc.vector.tensor_copy(b22b, b22)
            f1s, fc_b1 = [], []
            for h in range(2):
                t = wide.tile([128, 128], BF16, tag=f"f1s{h}")
                nc.vector.tensor_scalar(out=t, in0=W['f1'][h], scalar1=sc2,
                                        scalar2=None, op0=OP.mult)
                f1s.append(t)
                pb1 = ps_b.tile([128, 512], F32, tag="pb")
                nc.tensor.matmul(pb1[:, 0:1], W['f1'][h], b22b,
                                 start=True, stop=True)
                bb1 = smalls.tile([128, 1], F32, tag=f"fcb{h}")
                nc.vector.tensor_tensor(out=bb1, in0=pb1[:, 0:1],
                                        in1=W['f1b'][h], op=OP.add)
                fc_b1.append(bb1)
            for bb in range(2):
                fbb = o2f[:, bb * 384:(bb + 1) * 384]
                e1 = []
                for h in range(2):
                    pp = ps_a.tile([128, 512], F32, tag="pa")
                    nc.tensor.matmul(pp[:, 0:384], f1s[h], fbb,
                                     start=True, stop=True)
                    e1t = wide.tile([128, 384], BF16, tag=f"e1{h}")
                    if h == 0:
                        nc.scalar.activation(out=e1t, in_=pp[:, 0:384],
                                             func=AF.Relu, bias=fc_b1[h],
                                             scale=1.0)
                    else:
                        nc.vector.tensor_scalar(out=e1t, in0=pp[:, 0:384],
                                                scalar1=fc_b1[h],
                                                scalar2=0.0, op0=OP.add,
                                                op1=OP.max)
                    e1.append(e1t)
                for h in range(2):
                    pp = ps_a.tile([128, 512], F32, tag="pa")
                    nc.tensor.matmul(pp[:, 0:384], W['f2'][h][0], e1[0],
                                     start=True, stop=False)
                    nc.tensor.matmul(pp[:, 0:384], W['f2'][h][1], e1[1],
                                     start=False, stop=True)
                    mx = smalls.tile([128, 1], F32, tag="mx")
                    nc.vector.tensor_reduce(out=mx, in_=pp[:, 0:384],
                                            axis=mybir.AxisListType.X,
                                            op=OP.max)
                    nc.vector.tensor_scalar(out=om[:, bb, h:h + 1], in0=mx,
                                            scalar1=W['f2b'][h], scalar2=None,
                                            op0=OP.add)
            dst = bass.AP(tensor=out_d[:].tensor, offset=out_d[:].offset,
                          ap=[[1, 128], [256, 2], [128, 2]])
            nc.sync.dma_start(out=dst, in_=om)

    nc.compile()
    _CACHE[variant] = nc
    return nc


def _prep_inputs(inputs):
    """Host-side constant relayout + per-core slicing. Returns in_maps list."""
    xyz = _f32(inputs["xyz"])          # [2, 384, 3]
    feats = _f32(inputs["feats"])      # [2, 384, 1]

    Wq, Wk, Wv = inputs["tb_Wq"], inputs["tb_Wk"], inputs["tb_Wv"]
    Wg1, bg1 = inputs["tb_Wg1"], inputs["tb_bg1"]
    Wg2, bg2 = inputs["tb_Wg2"], inputs["tb_bg2"]
    Wpe, bpe = inputs["tb_Wpe"], inputs["tb_bpe"]

    L_nWk = np.zeros((NB, 128, 128), np.float32)
    L_Wv = np.zeros((NB, 128, 128), np.float32)
    L_G1Q = np.zeros((NB, 128, 128), np.float32)
    L_Wg1 = np.zeros((NB, 128, 128), np.float32)
    L_Wg2 = np.zeros((NB, 128, 128), np.float32)
    L_G1P8 = np.zeros((NB, 128, 128), np.float32)
    L_PsI8 = np.zeros((NB, 4, 128, 256), np.float32)
    sp3 = np.zeros((3, NB * 3 * 128), np.float32)
    sp1 = np.zeros((1, NB * 2 * 128), np.float32)
    I128 = np.eye(128, dtype=np.float32)
    for i in range(NB):
        Ws2, Wd = _wpe_split(_f32(Wpe[i]))
        g1 = _f32(Wg1[i])
        L_nWk[i] = (-_f32(Wk[i])).T
        L_Wv[i] = _f32(Wv[i]).T
        L_G1Q[i] = (g1 @ _f32(Wq[i])).T
        L_Wg1[i] = g1.T
        L_Wg2[i] = _f32(Wg2[i]).T
        G1P = (g1 @ Ws2).T                      # [30, 128]
        PsT = Ws2.T                             # [30, 128]
        for s in range(4):
            L_G1P8[i, 32 * s:32 * s + 30, :] = G1P
            L_PsI8[i, s, :, 0:128] = I128
            L_PsI8[i, s, 32 * s:32 * s + 30, 128:256] = PsT
        sp3[:, (i * 3 + 0) * 128:(i * 3 + 0) * 128 + 128] = (-4.0 * Wd).T
        sp3[:, (i * 3 + 1) * 128:(i * 3 + 1) * 128 + 128] = (4.0 * (g1 @ Wd)).T
        sp3[:, (i * 3 + 2) * 128:(i * 3 + 2) * 128 + 128] = (4.0 * Wd).T
        sp1[0, (2 * i) * 128:(2 * i) * 128 + 128] = g1 @ _f32(bpe[i]) + _f32(bg1[i])
        sp1[0, (2 * i + 1) * 128:(2 * i + 1) * 128 + 128] = _f32(bpe[i])

    vec_pack = np.zeros((128, 24), np.float32)
    for i in range(NB):
        vec_pack[:, i] = _f32(bg2[i])
        vec_pack[:, 3 + i] = _f32(inputs["tb_gamma"][i])
        vec_pack[:, 6 + i] = _f32(inputs["tb_beta"][i])
    vec_pack[:, 9] = _f32(inputs["enc_b"])
    for j in range(NF):
        vec_pack[:, 10 + j] = _f32(inputs["em_b1"][j])
        vec_pack[:, 12 + j] = _f32(inputs["em_b2"][j])
        vec_pack[:, 14 + j] = _f32(inputs["em_gamma"][j])
        vec_pack[:, 16 + j] = _f32(inputs["em_beta"][j])
    W1f = _f32(inputs["fcf_W1"])
    vec_pack[:, 18] = _f32(inputs["fcf_b1"])[0:128]
    vec_pack[:, 19] = _f32(inputs["fcf_b1"])[128:256]
    vec_pack[:, 20] = _f32(inputs["fcf_b2"])[0:128]
    vec_pack[:, 21] = _f32(inputs["fcf_b2"])[128:256]
    vec_pack[:, 22] = np.full(128, np.frombuffer(
        MAGIC.tobytes(), dtype=np.float32)[0], np.float32)
    vec_pack[:, 23] = -np.pi

    W2 = _f32(inputs["fcf_W2"])
    L_f2 = np.zeros((128, 4 * 128), np.float32)
    for h in range(2):
        for k in range(2):
            L_f2[:, (2 * h + k) * 128:(2 * h + k + 1) * 128] = \
                W2.T[k * 128:(k + 1) * 128, h * 128:(h + 1) * 128]

    com = {
        "feats_row": _bf(feats.reshape(1, 768)),
        "L_Wg1": _bf(L_Wg1), "L_Wg2": _bf(L_Wg2), "L_nWk": _bf(L_nWk),
        "L_Wv": _bf(L_Wv), "L_G1Q": _bf(L_G1Q),
        "L_G1P8": _f8(L_G1P8),
        "L_PsI8": _f8(L_PsI8.reshape(NB * 4, 128, 256)),
        "sp3": _bf(sp3), "sp1": _bf(sp1), "vec_pack": _f32(vec_pack),
        "L_enc": _bf(_f32(inputs["enc_W"])[:, 0:1].T),
        "L_em1": _bf(np.stack([_f32(inputs["em_W1"][j]).T for j in range(NF)])),
        "L_em2": _bf(np.stack([_f32(inputs["em_W2"][j]).T for j in range(NF)])),
        "L_f1": _bf(W1f.T.reshape(128, 2, 128).transpose(1, 0, 2)),
        "L_f2": _bf(L_f2),
    }

    in_maps = []
    for cix in range(8):
        b, qo = cix // 4, (cix % 4) * 96
        xk = xyz[b].T                      # [3, 384]
        S4 = np.zeros((4, NCH, 128), np.float32)
        for c in range(NCH):
            for qr in range(4):
                qg = qo + 4 * c + qr
                for j in range(3):
                    for t in range(10):
                        col = 32 * qr + 10 * j + t
                        s = np.float32(4.0 * FREQS[t % 5] / TWO_PI)
                        off = np.float32(0.25 if t >= 5 else 0.0)
                        S4[j, c, col] = s
                        S4[3, c, col] = (off + 512.0
                                         - s * np.float32(xyz[b, qg, j]))
        m = dict(com)
        m["xk_f"] = _f32(xk)
        m["s4_f"] = _f32(S4.reshape(4, NCH * 128))
        m["xk_b"] = _bf(xk)
        m["xq_b"] = _bf(xk[:, qo:qo + 96])
        m["feats_b"] = _bf(feats[b].reshape(1, 384))
        m["feats_q"] = _bf(feats[b, qo:qo + 96].reshape(1, 96))
        in_maps.append(m)
    return in_maps


def kernel(**inputs):
    from concourse.bass_utils import run_bass_kernel_spmd

    nc = _build()
    in_maps = _prep_inputs(inputs)
    res = run_bass_kernel_spmd(nc, in_maps, list(range(8)))
    return np.asarray(res.results[0]["out"], np.float32)


if __name__ == "__main__":
    print("smoke build only")


# revision 40
# speedup vs baseline: 1.0094x; 1.0071x over previous
"""NePuEncoder Bass/Tile kernel for 8 Trainium2 NeuronCores.

Sharding: query-parallel. Core c handles batch b=c//4, queries qo=(c%4)*96 ..
qo+96 of that batch. Channel-major layout [128 chan, keys] throughout.

Per-query attention fully fused in SBUF/PSUM:
  - PE:   hpre = G1P8(fp8,K=32)@trig8 + Wg1(bf16)@EK;  logits = Wg2@h;
          pos  = ONE fp8 DoubleRow matmul (I8 (x) VK8  +  PsShift8 (x) trig8)
          (per-query PE cost 560ns vs 800ns all-bf16)
  - relu(hpre+QB[m]): 1/3 on ACT, 2/3 on DVE (balance)
  - ACT:  w = exp(logits + bg2) with accum -> S0
  - DVE:  S1[m] = sum_n (pos + QP[m]) * w   (scalar_tensor_tensor)
Trig features are fp8 e4m3, built in stage 1: r = S4@[xk;1] (fp32 matmul),
round via +/-C trick (DVE), trig = Sin(2*pi*frac) written straight to fp8.
BN rsqrt via fast-inverse-sqrt bit hack + 2 Newton steps (keeps the ACT
exp table resident; no LoadActFuncSet churn).  The per-block AllGather is
split A(80 queries)/B(16+stats, stats ride as f32 bits in the bf16
payload); chunk A's collective+gather hide under the remaining queries.
BN affines are folded into the MLP/FC weights (per-channel scale on lhsT,
constant shift cancels in the next BN), so no full-width affine sits on
the boundary critical path.  Final FC runs on the gathered o2 directly.

HW-validated constraints: GPSIMD does no compute here (TensorScalarPtr /
PSUM access unsupported); tc.If branches contain only ACT ops (DVE ops
inside If/Else crash the device); DMAs issue on sync/gpsimd queues only;
AluOpType.mod is not valid TensorScalar ISA.
"""
import sys

sys.path.insert(0, "/opt/trn_rl_repo")

import numpy as np
import ml_dtypes

B, N, D, DS, LAT, FD, NF = 2, 384, 128, 3, 256, 1, 2
NB = NF + 1
NQ = 96                  # queries per core
NCH = 24                 # stage-1 chunks (4 queries each)
QA = 80                  # chunk-A query count for the split collective
FREQS = np.linspace(1.0, 32.0, 5).astype(np.float64)
EPS = 1e-5
TWO_PI = float(2 * np.pi)
MAGIC = np.uint32(0x5F3759DF)
C_ROUND = float(3 << 22)  # fp32 round-to-nearest-even trick

BF = ml_dtypes.bfloat16
F8NP = ml_dtypes.float8_e4m3


def _bf(x):
    return np.ascontiguousarray(np.asarray(x, np.float32).astype(BF))


def _f8(x):
    return np.ascontiguousarray(np.asarray(x, np.float32).astype(F8NP))


def _f32(x):
    return np.ascontiguousarray(np.asarray(x, np.float32))


def _wpe_split(Wpe):
    """Ws2 [128,30] trig cols with device sign (-sin trick) folded in:
    row r=10j+t: t<5 -> +Wpe[:,3+6t+j] (sin), t>=5 -> -Wpe[:,3+6(t-5)+3+j]."""
    Ws2 = np.zeros((D, 30), np.float32)
    for j in range(3):
        for t in range(10):
            r = 10 * j + t
            if t < 5:
                Ws2[:, r] = -Wpe[:, 3 + 6 * t + j]
            else:
                Ws2[:, r] = Wpe[:, 3 + 6 * (t - 5) + 3 + j]
    return Ws2, Wpe[:, 0:3].astype(np.float32)


_CACHE = {}


def _build(variant="spmd"):
    if variant in _CACHE:
        return _CACHE[variant]

    import concourse.bacc as bacc
    import concourse.bass as bass
    import concourse.tile as tile
    from concourse import mybir

    F32, BF16 = mybir.dt.float32, mybir.dt.bfloat16
    F8 = mybir.dt.float8e4
    U32 = mybir.dt.uint32
    AF = mybir.ActivationFunctionType
    OP = mybir.AluOpType
    PM = mybir.MatmulPerfMode

    nc = bacc.Bacc(None, target_bir_lowering=False,
                   num_devices=(8 if variant == "spmd" else 1))

    def din(name, shape, dt=BF16):
        return nc.dram_tensor(name, shape, dt, kind="ExternalInput")

    # per-core inputs
    xk_f = din("xk_f", [3, 384], F32)
    s4_f = din("s4_f", [4, NCH * 128], F32)
    xk_b = din("xk_b", [3, 384])
    xq_b = din("xq_b", [3, 96])
    feats_b = din("feats_b", [1, 384])
    feats_q = din("feats_q", [1, 96])
    # replicated inputs
    feats_row = din("feats_row", [1, 768])
    L_Wg1 = din("L_Wg1", [NB, 128, 128])
    L_Wg2 = din("L_Wg2", [NB, 128, 128])
    L_nWk = din("L_nWk", [NB, 128, 128])
    L_Wv = din("L_Wv", [NB, 128, 128])
    L_G1Q = din("L_G1Q", [NB, 128, 128])
    L_G1P8 = din("L_G1P8", [NB, 128, 128], F8)
    L_PsI8 = din("L_PsI8", [NB * 4, 128, 256], F8)
    sp3 = din("sp3", [3, NB * 3 * 128])       # nPd4 | G1Pd4 | Pd4 per block
    sp1 = din("sp1", [1, NB * 2 * 128])       # c1 | bpe per block
    vec_pack = din("vec_pack", [128, 24], F32)
    L_enc = din("L_enc", [1, 128])
    L_em1 = din("L_em1", [NF, 128, 128])
    L_em2 = din("L_em2", [NF, 128, 128])
    L_f1 = din("L_f1", [2, 128, 128])
    L_f2 = din("L_f2", [128, 4 * 128])        # f2[h,k] at col (2h+k)*128

    out_d = nc.dram_tensor("out", [2, 256], F32, kind="ExternalOutput")
    RG = [[0, 1, 2, 3, 4, 5, 6, 7]]

    with tile.TileContext(nc) as tc:
        with (
            tc.tile_pool(name="sing", bufs=1) as sing,
            tc.tile_pool(name="fpool", bufs=2) as fpool,
            tc.tile_pool(name="blk", bufs=2) as blk,
            tc.tile_pool(name="hp", bufs=6) as hp,
            tc.tile_pool(name="wp", bufs=6) as wp,
            tc.tile_pool(name="st1", bufs=2) as st1,
            tc.tile_pool(name="wide", bufs=2) as wide,
            tc.tile_pool(name="smalls", bufs=4) as smalls,
            tc.tile_pool(name="ps_a", bufs=2, space="PSUM") as ps_a,
            tc.tile_pool(name="ps_b", bufs=2, space="PSUM") as ps_b,
            tc.tile_pool(name="ps_g", bufs=4, space="PSUM") as ps_g,
            tc.tile_pool(name="dram", bufs=1, space="DRAM") as dram,
        ):
            def load(src, shape, dt=BF16, pool=sing, tag=None, q=None):
                t = pool.tile(shape, dt, tag=tag, name=tag or "ld")
                (q or nc.sync).dma_start(out=t, in_=src)
                return t

            def loadfam(srcT, nblk, tag, dt=BF16, w=128, q=None):
                t = sing.tile([128, nblk * w], dt, tag=tag, name=tag)
                ap = srcT[:]
                s = bass.AP(tensor=ap.tensor, offset=ap.offset,
                            ap=[[w, 128], [128 * w, nblk], [1, w]])
                (q or nc.sync).dma_start(
                    out=t.rearrange("p (i c) -> p i c", i=nblk), in_=s)
                return [t[:, i * w:(i + 1) * w] for i in range(nblk)]

            # ---- critical loads for stage 1 ----
            xko = sing.tile([4, 384], F32, tag="xko")
            nc.vector.memset(xko, 1.0)
            nc.sync.dma_start(out=xko[0:3, :], in_=xk_f[:])
            s4_sb = load(s4_f[:], [4, NCH * 128], F32, tag="s4")

            vp = load(vec_pack[:], [128, 24], F32, tag="vp")
            magic = vp[:, 22:23]
            negpi = vp[:, 23:24]

            # fp8 mega tile: [VK8 | 24 trig chunks] each 384 cols
            T8 = sing.tile([128, (1 + NCH) * 384], F8, tag="T8", name="T8")
            t8ap = T8[:]

            def t8_slot(c):
                return T8[:, (1 + c) * 384:(2 + c) * 384]

            # ---------- stage 1: trig via S4 matmul + round + Sin -------
            # even chunks round on DVE (frac), odd on ACT+DVE (-frac, sign
            # folded into the Sin scale)
            cC = sing.tile([128, 1], F32, tag="cC")
            nc.vector.memset(cC, C_ROUND)
            for c in range(NCH):
                rp = ps_a.tile([128, 512], F32, tag="pa")
                nc.tensor.matmul(rp[:, 0:384], s4_sb[:, c * 128:(c + 1) * 128],
                                 xko, start=True, stop=True)
                if c % 2 == 0:
                    n_t = st1.tile([128, 384], F32, tag="nt")
                    nc.vector.tensor_scalar(out=n_t, in0=rp[:, 0:384],
                                            scalar1=C_ROUND, scalar2=C_ROUND,
                                            op0=OP.add, op1=OP.subtract)
                    n_s = st1.tile([128, 384], F32, tag="ns")
                    nc.vector.tensor_tensor(out=n_s, in0=rp[:, 0:384],
                                            in1=n_t, op=OP.subtract)
                    nc.scalar.activation(out=t8_slot(c), in_=n_s, func=AF.Sin,
                                         bias=0.0, scale=TWO_PI)
                else:
                    rc = st1.tile([128, 384], F32, tag="rc")
                    nc.scalar.activation(out=rc, in_=rp[:, 0:384],
                                         func=AF.Identity, bias=cC, scale=1.0)
                    n_s = st1.tile([128, 384], F32, tag="ns")
                    nc.vector.scalar_tensor_tensor(
                        out=n_s, in0=rc, scalar=C_ROUND, in1=rp[:, 0:384],
                        op0=OP.subtract, op1=OP.subtract)
                    nc.scalar.activation(out=t8_slot(c), in_=n_s, func=AF.Sin,
                                         bias=0.0, scale=-TWO_PI)

            # ---- bulk loads (queue behind stage-1 issues) ----
            W = {}
            W['xkb'] = load(xk_b[:], [3, 384], tag="sxkb")
            W['xqb'] = load(xq_b[:], [3, 96], tag="sxqb")
            W['featsb'] = load(feats_b[:], [1, 384], tag="sfb")
            W['featsq'] = load(feats_q[:], [1, 96], tag="sfq")
            W['feats'] = load(feats_row[:], [1, 768], tag="sfr")
            W['Wg1'] = loadfam(L_Wg1, NB, "wg1")
            W['Wg2'] = loadfam(L_Wg2, NB, "wg2")
            W['nWk'] = loadfam(L_nWk, NB, "nwk")
            W['Wv'] = loadfam(L_Wv, NB, "wv", q=nc.gpsimd)
            W['G1Q'] = loadfam(L_G1Q, NB, "g1q", q=nc.gpsimd)
            W['G1P8'] = loadfam(L_G1P8, NB, "g1p8", dt=F8, q=nc.gpsimd)
            psi = loadfam(L_PsI8, NB * 4, "psi8", dt=F8, w=256, q=nc.gpsimd)
            W['PsI8'] = [[psi[4 * i + s].rearrange("p (s2 k) -> p s2 k", s2=2)
                          for s in range(4)] for i in range(NB)]
            sp3_sb = load(sp3[:], [3, NB * 3 * 128], tag="sp3", q=nc.gpsimd)
            sp1_sb = load(sp1[:], [1, NB * 2 * 128], tag="sp1", q=nc.gpsimd)
            W['enc'] = load(L_enc[:], [1, 128], tag="enc", q=nc.gpsimd)
            W['em1'] = loadfam(L_em1, NF, "em1", q=nc.gpsimd)
            W['em2'] = loadfam(L_em2, NF, "em2", q=nc.gpsimd)
            W['f1'] = loadfam(L_f1, 2, "f1", q=nc.gpsimd)
            f2all = load(L_f2[:], [128, 4 * 128], tag="f2", q=nc.gpsimd)
            W['f2'] = [[f2all[:, (2 * h + k) * 128:(2 * h + k + 1) * 128]
                        for k in range(2)] for h in range(2)]

            def blkslice(base, i, j, w3):
                return base[:, (i * 3 + j) * 128:(i * 3 + j) * 128 + 128]

            W['nPd4'] = [sp3_sb[:, (i * 3 + 0) * 128:(i * 3 + 0) * 128 + 128]
                         for i in range(NB)]
            W['G1Pd4'] = [sp3_sb[:, (i * 3 + 1) * 128:(i * 3 + 1) * 128 + 128]
                          for i in range(NB)]
            W['Pd4'] = [sp3_sb[:, (i * 3 + 2) * 128:(i * 3 + 2) * 128 + 128]
                        for i in range(NB)]
            W['c1'] = [sp1_sb[:, (2 * i) * 128:(2 * i) * 128 + 128]
                       for i in range(NB)]
            W['bpe'] = [sp1_sb[:, (2 * i + 1) * 128:(2 * i + 1) * 128 + 128]
                        for i in range(NB)]
            W['bg2'] = [vp[:, i:i + 1] for i in range(NB)]
            W['gam'] = [vp[:, 3 + i:4 + i] for i in range(NB)]
            W['bet'] = [vp[:, 6 + i:7 + i] for i in range(NB)]
            W['encb'] = vp[:, 9:10]
            W['emb1'] = [vp[:, 10 + j:11 + j] for j in range(NF)]
            W['emb2'] = [vp[:, 12 + j:13 + j] for j in range(NF)]
            W['emg'] = [vp[:, 14 + j:15 + j] for j in range(NF)]
            W['embe'] = [vp[:, 16 + j:17 + j] for j in range(NF)]
            W['f1b'] = [vp[:, 18 + h:19 + h] for h in range(2)]
            W['f2b'] = [vp[:, 20 + h:21 + h] for h in range(2)]
            magic = vp[:, 22:23]
            negpi = vp[:, 23:24]

            ones96 = sing.tile([1, 96], BF16, tag="ones96")
            nc.vector.memset(ones96, 1.0)
            ones384 = sing.tile([128, 384], BF16, tag="ones384")
            nc.vector.memset(ones384, 1.0)
            dumA = sing.tile([128, 1], BF16, tag="dumA")

            # ---------- initial features (ps_b: overlaps stage-1) ----------
            fb = fpool.tile([128, 384], BF16, tag="fb")
            p = ps_b.tile([128, 512], F32, tag="pb")
            nc.tensor.matmul(p[:, 0:384], W['enc'], W['featsb'], start=True,
                             stop=True)
            nc.scalar.activation(out=fb, in_=p[:, 0:384], func=AF.Identity,
                                 bias=W['encb'], scale=1.0)
            fq = fpool.tile([128, 96], BF16, tag="fq")
            p = ps_b.tile([128, 512], F32, tag="pb")
            nc.tensor.matmul(p[:, 0:96], W['enc'], W['featsq'], start=True,
                             stop=True)
            nc.scalar.activation(out=fq, in_=p[:, 0:96], func=AF.Identity,
                                 bias=W['encb'], scale=1.0)

            pid = nc.scalar.partition_id()

            def rsqrt(var_ap, tag):
                """y ~ 1/sqrt(var + EPS): bit hack + 2 Newton steps (DVE)."""
                u = smalls.tile([128, 1], F32, tag="rq_u")
                nc.vector.tensor_scalar(out=u, in0=var_ap, scalar1=EPS,
                                        scalar2=None, op0=OP.add)
                h = smalls.tile([128, 1], U32, tag="rq_h")
                nc.vector.tensor_scalar(out=h, in0=u[:].bitcast(U32),
                                        scalar1=1, scalar2=None,
                                        op0=OP.logical_shift_right)
                y = smalls.tile([128, 1], F32, tag=tag, name=tag)
                nc.vector.tensor_tensor(out=y[:].bitcast(U32),
                                        in0=magic.bitcast(U32), in1=h,
                                        op=OP.subtract)
                for _ in range(2):
                    t1 = smalls.tile([128, 1], F32, tag="rq_t1")
                    nc.vector.tensor_tensor(out=t1, in0=y, in1=y, op=OP.mult)
                    nc.vector.tensor_tensor(out=t1, in0=u, in1=t1, op=OP.mult)
                    nc.vector.tensor_scalar(out=t1, in0=t1, scalar1=-0.5,
                                            scalar2=1.5, op0=OP.mult,
                                            op1=OP.add)
                    y2 = smalls.tile([128, 1], F32, tag=tag, name=tag)
                    nc.vector.tensor_tensor(out=y2, in0=y, in1=t1, op=OP.mult)
                    y = y2
                return y

            # ---------- transformer blocks ----------
            for i in range(NB):
                # block consts: EK(bf16), VK8(fp8), QB, QP  (ps_b ring)
                pa = ps_b.tile([128, 512], F32, tag="pb")
                nc.tensor.matmul(pa[:, 0:384], W['nWk'][i], fb, start=True,
                                 stop=False)
                nc.tensor.matmul(pa[:, 0:384], W['nPd4'][i][0:3, :], W['xkb'],
                                 start=False, stop=True)
                EK = blk.tile([128, 384], BF16, tag="EK")
                nc.scalar.copy(EK, pa[:, 0:384])

                pb = ps_b.tile([128, 512], F32, tag="pb")
                nc.tensor.matmul(pb[:, 0:384], W['Wv'][i], fb, start=True,
                                 stop=False)
                nc.tensor.matmul(pb[:, 0:384], W['nPd4'][i][0:3, :], W['xkb'],
                                 start=False, stop=True)
                nc.scalar.copy(T8[:, 0:384], pb[:, 0:384])   # VK8 fp8

                pa = ps_b.tile([128, 512], F32, tag="pb")
                nc.tensor.matmul(pa[:, 0:96], W['G1Q'][i], fq, start=True,
                                 stop=False)
                nc.tensor.matmul(pa[:, 0:96], W['G1Pd4'][i][0:3, :], W['xqb'],
                                 start=False, stop=False)
                nc.tensor.matmul(pa[:, 0:96], W['c1'][i][0:1, :], ones96,
                                 start=False, stop=True)
                QB = blk.tile([128, 96], F32, tag="QB")
                nc.vector.tensor_copy(QB, pa[:, 0:96])

                pb = ps_b.tile([128, 512], F32, tag="pb")
                nc.tensor.matmul(pb[:, 0:96], W['Pd4'][i][0:3, :], W['xqb'],
                                 start=True, stop=False)
                nc.tensor.matmul(pb[:, 0:96], W['bpe'][i][0:1, :], ones96,
                                 start=False, stop=True)
                QP = blk.tile([128, 96], F32, tag="QP")
                nc.vector.tensor_copy(QP, pb[:, 0:96])

                S1 = blk.tile([128, 96], F32, tag="S1")
                S0 = blk.tile([128, 96], F32, tag="S0")
                # payload: 96 bf16 o-cols + 2 f32 stats (as raw bits)
                P16 = blk.tile([128, 100], BF16, tag="P16")
                P32 = P16[:].bitcast(F32)          # [128, 50]

                ag_in_A = dram.tile([128, QA], BF16, tag=f"aginA{i}")
                ag_in_B = dram.tile([128, 100 - QA], BF16, tag=f"aginB{i}")
                shr = dict(addr_space="Shared") if variant == "spmd" else {}
                ag_out_A = dram.tile([8, 128, QA], BF16, tag=f"agoutA{i}",
                                     **shr)
                ag_out_B = dram.tile([8, 128, 100 - QA], BF16,
                                     tag=f"agoutB{i}", **shr)

                def fan(ag_in, ag_out, src_ap, queues):
                    nc.sync.dma_start(out=ag_in, in_=src_ap)
                    if variant == "spmd":
                        nc.gpsimd.collective_compute(
                            "AllGather", OP.bypass, replica_groups=RG,
                            ins=[ag_in[:].opt()], outs=[ag_out[:].opt()])
                    else:
                        for cc in range(8):
                            queues[cc % len(queues)].dma_start(
                                out=ag_out[cc], in_=src_ap)

                # gathered tile: [p, core, 100] bf16 (o cols 0:96 + stat bits)
                o_ext = wide.tile([128, 8, 100], BF16, tag="oext")
                oe = o_ext[:]
                oe32 = oe.bitcast(F32)             # [p, 8, 50]

                def o_half(h):
                    """[128, 384] bf16 view: cores 4h..4h+4, o cols 0:96."""
                    return o_ext[:, 4 * h:4 * h + 4, 0:96]

                # attention, 1-query software pipeline: relu(m+1) is
                # emitted before stt(m) so DVE never head-of-line blocks;
                # relu 1/3 ACT 2/3 DVE, exp+accum ACT, S1 stt DVE, fp8 DR pos
                def hpre_relu(m):
                    c, s = m // 4, m % 4
                    tsl8 = T8[32 * s:32 * s + 32,
                              (1 + c) * 384:(1 + c) * 384 + 384]
                    pa = ps_a.tile([128, 512], F32, tag="pa")
                    nc.tensor.matmul(pa[:, 0:384],
                                     W['G1P8'][i][32 * s:32 * s + 32, :],
                                     tsl8, start=True, stop=False,
                                     tile_position=(32 * s, 0))
                    nc.tensor.matmul(pa[:, 0:384], W['Wg1'][i], EK,
                                     start=False, stop=True)
                    h_t = hp.tile([128, 384], BF16, tag="h")
                    if m % 3 == 0:
                        nc.scalar.activation(out=h_t, in_=pa[:, 0:384],
                                             func=AF.Relu,
                                             bias=QB[:, m:m + 1], scale=1.0)
                    else:
                        nc.vector.tensor_scalar(
                            out=h_t, in0=pa[:, 0:384],
                            scalar1=QB[:, m:m + 1],
                            scalar2=0.0, op0=OP.add, op1=OP.max)
                    return h_t

                hq = [hpre_relu(0), hpre_relu(1)]
                for m in range(NQ):
                    c, s = m // 4, m % 4
                    lg = ps_g.tile([128, 512], F32, tag="lg")
                    nc.tensor.matmul(lg[:, 0:384], W['Wg2'][i], hq.pop(0),
                                     start=True, stop=True)
                    if m + 2 < NQ:
                        hq.append(hpre_relu(m + 2))
                    w_t = wp.tile([128, 384], BF16, tag="w")
                    nc.scalar.activation(out=w_t, in_=lg[:, 0:384],
                                         func=AF.Exp, bias=W['bg2'][i],
                                         scale=1.0, accum_out=S0[:, m:m + 1])
                    pos = ps_b.tile([128, 512], F32, tag="pb")
                    rhs = bass.AP(tensor=t8ap.tensor, offset=t8ap.offset,
                                  ap=[list(t8ap.ap[0]), [(1 + c) * 384, 2],
                                      [1, 384]])
                    nc.tensor.matmul(pos[:, 0:384], W['PsI8'][i][s], rhs,
                                     start=True, stop=True,
                                     perf_mode=PM.DoubleRow)
                    nc.vector.scalar_tensor_tensor(
                        out=dumA.broadcast_to((128, 384)),
                        in0=pos[:, 0:384], scalar=QP[:, m:m + 1],
                        in1=w_t, op0=OP.add, op1=OP.mult,
                        accum_out=S1[:, m:m + 1])

                    if m == QA + 3:
                        # chunk A payload + collective (hidden under B's
                        # attention)
                        R = smalls.tile([128, QA], F32, tag="RA")
                        nc.vector.reciprocal(out=R, in_=S0[:, 0:QA])
                        nc.vector.tensor_tensor(out=R, in0=S1[:, 0:QA],
                                                in1=R, op=OP.mult)
                        nc.vector.tensor_tensor(out=P16[:, 0:QA], in0=R,
                                                in1=fq[:, 0:QA], op=OP.add)
                        fan(ag_in_A, ag_out_A, P16[:, 0:QA], [nc.sync])
                        src = bass.AP(
                            tensor=ag_out_A[:].tensor,
                            offset=ag_out_A[:].offset,
                            ap=[[QA, 128], [128 * QA, 8], [1, QA]])
                        nc.sync.dma_start(out=o_ext[:, :, 0:QA], in_=src)

                # chunk B payload + stats
                R = smalls.tile([128, 16], F32, tag="RB")
                nc.vector.reciprocal(out=R, in_=S0[:, QA:96])
                nc.vector.tensor_tensor(out=R, in0=S1[:, QA:96], in1=R,
                                        op=OP.mult)
                nc.vector.tensor_tensor(out=P16[:, QA:96], in0=R,
                                        in1=fq[:, QA:96], op=OP.add)
                st = smalls.tile([128, 6], F32, tag="bnst")
                nc.vector.bn_stats(out=st, in_=P16[:, 0:96])
                mv = smalls.tile([128, 2], F32, tag="bnmv")
                nc.vector.bn_aggr(out=mv, in_=st)
                nc.vector.tensor_copy(P32[:, 48:49], mv[:, 0:1])
                msq = smalls.tile([128, 1], F32, tag="msq")
                nc.vector.tensor_tensor(out=msq, in0=mv[:, 0:1],
                                        in1=mv[:, 0:1], op=OP.mult)
                nc.vector.tensor_tensor(out=P32[:, 49:50], in0=mv[:, 1:2],
                                        in1=msq, op=OP.add)
                fan(ag_in_B, ag_out_B, P16[:, QA:100],
                    [nc.sync, nc.gpsimd, nc.sync, nc.gpsimd, nc.sync,
                     nc.gpsimd, nc.sync, nc.sync])
                srcB = bass.AP(tensor=ag_out_B[:].tensor,
                               offset=ag_out_B[:].offset,
                               ap=[[100 - QA, 128], [128 * (100 - QA), 8],
                                   [1, 100 - QA]])
                nc.sync.dma_start(out=o_ext[:, :, QA:100], in_=srcB)

                mg = smalls.tile([128, 1], F32, tag="mg")
                stm = bass.AP(tensor=oe32.tensor, offset=oe32.offset + 48,
                              ap=[list(oe32.ap[0]), [50, 8]])
                nc.vector.tensor_reduce(out=mg, in_=stm,
                                        axis=mybir.AxisListType.X, op=OP.add)
                nc.vector.tensor_scalar(out=mg, in0=mg, scalar1=0.125,
                                        scalar2=None, op0=OP.mult)
                e2g = smalls.tile([128, 1], F32, tag="e2g")
                ste = bass.AP(tensor=oe32.tensor, offset=oe32.offset + 49,
                              ap=[list(oe32.ap[0]), [50, 8]])
                nc.vector.tensor_reduce(out=e2g, in_=ste,
                                        axis=mybir.AxisListType.X, op=OP.add)
                nc.vector.tensor_scalar(out=e2g, in0=e2g, scalar1=0.125,
                                        scalar2=None, op0=OP.mult)
                var = smalls.tile([128, 1], F32, tag="var")
                nc.vector.tensor_tensor(out=var, in0=mg, in1=mg, op=OP.mult)
                nc.vector.tensor_tensor(out=var, in0=e2g, in1=var,
                                        op=OP.subtract)
                rs = rsqrt(var, "rs")
                sc = smalls.tile([128, 1], F32, tag="sc")
                nc.vector.tensor_tensor(out=sc, in0=W['gam'][i], in1=rs,
                                        op=OP.mult)
                b2 = smalls.tile([128, 1], F32, tag="b2")
                nc.vector.tensor_scalar(out=b2, in0=mg, scalar1=sc,
                                        scalar2=None, op0=OP.mult)
                nc.vector.tensor_tensor(out=b2, in0=W['bet'][i], in1=b2,
                                        op=OP.subtract)

                if i < NB - 1:
                    fq = fpool.tile([128, 96], BF16, tag="fq")
                    nc.vector.tensor_scalar(out=fq, in0=P16[:, 0:96],
                                            scalar1=sc, scalar2=b2,
                                            op0=OP.mult, op1=OP.add)
                if i == NB - 1:
                    fb = None                     # last block: no next EK/VK
                else:
                    fb = fpool.tile([128, 384], BF16, tag="fb")
                    with tc.If(pid < 4) as cmp:
                        nc.scalar.activation(out=fb, in_=o_half(0),
                                             func=AF.Identity, bias=b2,
                                             scale=sc)
                    with cmp.Else():
                        nc.scalar.activation(out=fb, in_=o_half(1),
                                             func=AF.Identity, bias=b2,
                                             scale=sc)

                # ---------- MLP with BN1 affine folded into layer 1 ----------
                if i > 0:
                    j = i - 1
                    # W1' = W1 * diag(sc)  (scale lhsT rows);  bias1' =
                    # W1 @ b2 + em_b1.  The +b2 shift of the residual input
                    # cancels inside BN2, so o2' = sc*o + y2 suffices.
                    em1s = wide.tile([128, 128], BF16, tag="em1s")
                    nc.vector.tensor_scalar(out=em1s, in0=W['em1'][j],
                                            scalar1=sc, scalar2=None,
                                            op0=OP.mult)
                    b2b = smalls.tile([128, 1], BF16, tag="b2b")
                    nc.vector.tensor_copy(b2b, b2)
                    pb1 = ps_b.tile([128, 512], F32, tag="pb")
                    nc.tensor.matmul(pb1[:, 0:1], W['em1'][j], b2b,
                                     start=True, stop=True)
                    bias1 = smalls.tile([128, 1], F32, tag="bias1")
                    nc.vector.tensor_tensor(out=bias1, in0=pb1[:, 0:1],
                                            in1=W['emb1'][j], op=OP.add)

                    def mlp_layer(lw, bias_ap, xins, width, tag):
                        t = wide.tile([128, width], BF16, tag=tag)
                        for hh, xin in enumerate(xins):
                            wdt = min(384, width - hh * 384)
                            pp = ps_a.tile([128, 512], F32, tag="pa")
                            nc.tensor.matmul(pp[:, 0:wdt], lw, xin,
                                             start=True, stop=True)
                            nc.scalar.activation(
                                out=t[:, hh * 384:hh * 384 + wdt],
                                in_=pp[:, 0:wdt], func=AF.Relu,
                                bias=bias_ap, scale=1.0)
                        return t

                    y1f = mlp_layer(em1s, bias1, [o_half(0), o_half(1)],
                                    768, "y1f")
                    y2f = mlp_layer(W['em2'][j], W['emb2'][j],
                                    [y1f[:, 0:384], y1f[:, 384:768]],
                                    768, "y2f")
                    o2f = wide.tile([128, 768], BF16, tag="o2f")
                    nc.vector.scalar_tensor_tensor(
                        out=o2f[:, 0:384], in0=o_half(0), scalar=sc,
                        in1=y2f[:, 0:384], op0=OP.mult, op1=OP.add)
                    nc.vector.scalar_tensor_tensor(
                        out=o2f[:, 384:768], in0=o_half(1), scalar=sc,
                        in1=y2f[:, 384:768], op0=OP.mult, op1=OP.add)
                    if i < NB - 1:
                        y1q = mlp_layer(em1s, bias1, [P16[:, 0:96]], 96,
                                        "y1q")
                        y2q = mlp_layer(W['em2'][j], W['emb2'][j], [y1q], 96,
                                        "y2q")
                        o2q = wide.tile([128, 96], BF16, tag="o2q")
                        nc.vector.scalar_tensor_tensor(
                            out=o2q, in0=P16[:, 0:96], scalar=sc, in1=y2q,
                            op0=OP.mult, op1=OP.add)

                    st2 = smalls.tile([128, 2, 6], F32, tag="st2")
                    nc.vector.bn_stats(out=st2[:, 0, :], in_=o2f[:, 0:384])
                    nc.vector.bn_stats(out=st2[:, 1, :], in_=o2f[:, 384:768])
                    mv2 = smalls.tile([128, 2], F32, tag="mv2")
                    nc.vector.bn_aggr(out=mv2, in_=st2)
                    rs2 = rsqrt(mv2[:, 1:2], "rs2")
                    sc2 = smalls.tile([128, 1], F32, tag="sc")
                    nc.vector.tensor_tensor(out=sc2, in0=W['emg'][j], in1=rs2,
                                            op=OP.mult)
                    b22 = smalls.tile([128, 1], F32, tag="b2")
                    nc.vector.tensor_scalar(out=b22, in0=mv2[:, 0:1],
                                            scalar1=sc2, scalar2=None,
                                            op0=OP.mult)
                    nc.vector.tensor_tensor(out=b22, in0=W['embe'][j],
                                            in1=b22, op=OP.subtract)
                    if i < NB - 1:
                        fq = fpool.tile([128, 96], BF16, tag="fq")
                        nc.vector.tensor_scalar(out=fq, in0=o2q, scalar1=sc2,
                                                scalar2=b22, op0=OP.mult,
                                                op1=OP.add)
                    if i < NB - 1:
                        fb = fpool.tile([128, 384], BF16, tag="fb")
                        with tc.If(pid < 4) as cmp:
                            nc.scalar.activation(out=fb, in_=o2f[:, 0:384],
                                                 func=AF.Identity, bias=b22,
                                                 scale=sc2)
                        with cmp.Else():
                            nc.scalar.activation(out=fb, in_=o2f[:, 384:768],
                                                 func=AF.Identity, bias=b22,
                                                 scale=sc2)

            # ---------- final FC + max (BN2 affine folded into f1) ----------
            om = smalls.tile([128, 2, 2], F32, tag="om")
            b22b = smalls.tile([128, 1], BF16, tag="b22b")
            nc.vector.tensor_copy(b22b, b22)
            f1s, fc_b1 = [], []
            for h in range(2):
                t = wide.tile([128, 128], BF16, tag=f"f1s{h}")
                nc.vector.tensor_scalar(out=t, in0=W['f1'][h], scalar1=sc2,
                                        scalar2=None, op0=OP.mult)
                f1s.append(t)
                pb1 = ps_b.tile([128, 512], F32, tag="pb")
                nc.tensor.matmul(pb1[:, 0:1], W['f1'][h], b22b,
                                 start=True, stop=True)
                bb1 = smalls.tile([128, 1], F32, tag=f"fcb{h}")
                nc.vector.tensor_tensor(out=bb1, in0=pb1[:, 0:1],
                                        in1=W['f1b'][h], op=OP.add)
                fc_b1.append(bb1)
            for bb in range(2):
                fbb = o2f[:, bb * 384:(bb + 1) * 384]
                e1 = []
                for h in range(2):
                    pp = ps_a.tile([128, 512], F32, tag="pa")
                    nc.tensor.matmul(pp[:, 0:384], f1s[h], fbb,
                                     start=True, stop=True)
                    e1t = wide.tile([128, 384], BF16, tag=f"e1{h}")
                    if h == 0:
                        nc.scalar.activation(out=e1t, in_=pp[:, 0:384],
                                             func=AF.Relu, bias=fc_b1[h],
                                             scale=1.0)
                    else:
                        nc.vector.tensor_scalar(out=e1t, in0=pp[:, 0:384],
                                                scalar1=fc_b1[h],
                                                scalar2=0.0, op0=OP.add,
                                                op1=OP.max)
                    e1.append(e1t)
                for h in range(2):
                    pp = ps_a.tile([128, 512], F32, tag="pa")
                    nc.tensor.matmul(pp[:, 0:384], W['f2'][h][0], e1[0],
                                     start=True, stop=False)
                    nc.tensor.matmul(pp[:, 0:384], W['f2'][h][1], e1[1],
                                     start=False, stop=True)
                    mx = smalls.tile([128, 1], F32, tag="mx")
                    nc.vector.tensor_reduce(out=mx, in_=pp[:, 0:384],
                                            axis=mybir.AxisListType.X,
                                            op=OP.max)
                    nc.vector.tensor_scalar(out=om[:, bb, h:h + 1], in0=mx,
                                            scalar1=W['f2b'][h], scalar2=None,
                                            op0=OP.add)
            dst = bass.AP(tensor=out_d[:].tensor, offset=out_d[:].offset,
                          ap=[[1, 128], [256, 2], [128, 2]])
            nc.sync.dma_start(out=dst, in_=om)

    nc.compile()
    _CACHE[variant] = nc
    return nc


def _prep_inputs(inputs):
    """Host-side constant relayout + per-core slicing. Returns in_maps list."""
    xyz = _f32(inputs["xyz"])          # [2, 384, 3]
    feats = _f32(inputs["feats"])      # [2, 384, 1]

    Wq, Wk, Wv = inputs["tb_Wq"], inputs["tb_Wk"], inputs["tb_Wv"]
    Wg1, bg1 = inputs["tb_Wg1"], inputs["tb_bg1"]
    Wg2, bg2 = inputs["tb_Wg2"], inputs["tb_bg2"]
    Wpe, bpe = inputs["tb_Wpe"], inputs["tb_bpe"]

    L_nWk = np.zeros((NB, 128, 128), np.float32)
    L_Wv = np.zeros((NB, 128, 128), np.float32)
    L_G1Q = np.zeros((NB, 128, 128), np.float32)
    L_Wg1 = np.zeros((NB, 128, 128), np.float32)
    L_Wg2 = np.zeros((NB, 128, 128), np.float32)
    L_G1P8 = np.zeros((NB, 128, 128), np.float32)
    L_PsI8 = np.zeros((NB, 4, 128, 256), np.float32)
    sp3 = np.zeros((3, NB * 3 * 128), np.float32)
    sp1 = np.zeros((1, NB * 2 * 128), np.float32)
    I128 = np.eye(128, dtype=np.float32)
    for i in range(NB):
        Ws2, Wd = _wpe_split(_f32(Wpe[i]))
        g1 = _f32(Wg1[i])
        L_nWk[i] = (-_f32(Wk[i])).T
        L_Wv[i] = _f32(Wv[i]).T
        L_G1Q[i] = (g1 @ _f32(Wq[i])).T
        L_Wg1[i] = g1.T
        L_Wg2[i] = _f32(Wg2[i]).T
        G1P = (g1 @ Ws2).T                      # [30, 128]
        PsT = Ws2.T                             # [30, 128]
        for s in range(4):
            L_G1P8[i, 32 * s:32 * s + 30, :] = G1P
            L_PsI8[i, s, :, 0:128] = I128
            L_PsI8[i, s, 32 * s:32 * s + 30, 128:256] = PsT
        sp3[:, (i * 3 + 0) * 128:(i * 3 + 0) * 128 + 128] = (-4.0 * Wd).T
        sp3[:, (i * 3 + 1) * 128:(i * 3 + 1) * 128 + 128] = (4.0 * (g1 @ Wd)).T
        sp3[:, (i * 3 + 2) * 128:(i * 3 + 2) * 128 + 128] = (4.0 * Wd).T
        sp1[0, (2 * i) * 128:(2 * i) * 128 + 128] = g1 @ _f32(bpe[i]) + _f32(bg1[i])
        sp1[0, (2 * i + 1) * 128:(2 * i + 1) * 128 + 128] = _f32(bpe[i])

    vec_pack = np.zeros((128, 24), np.float32)
    for i in range(NB):
        vec_pack[:, i] = _f32(bg2[i])
        vec_pack[:, 3 + i] = _f32(inputs["tb_gamma"][i])
        vec_pack[:, 6 + i] = _f32(inputs["tb_beta"][i])
    vec_pack[:, 9] = _f32(inputs["enc_b"])
    for j in range(NF):
        vec_pack[:, 10 + j] = _f32(inputs["em_b1"][j])
        vec_pack[:, 12 + j] = _f32(inputs["em_b2"][j])
        vec_pack[:, 14 + j] = _f32(inputs["em_gamma"][j])
        vec_pack[:, 16 + j] = _f32(inputs["em_beta"][j])
    W1f = _f32(inputs["fcf_W1"])
    vec_pack[:, 18] = _f32(inputs["fcf_b1"])[0:128]
    vec_pack[:, 19] = _f32(inputs["fcf_b1"])[128:256]
    vec_pack[:, 20] = _f32(inputs["fcf_b2"])[0:128]
    vec_pack[:, 21] = _f32(inputs["fcf_b2"])[128:256]
    vec_pack[:, 22] = np.full(128, np.frombuffer(
        MAGIC.tobytes(), dtype=np.float32)[0], np.float32)
    vec_pack[:, 23] = -np.pi

    W2 = _f32(inputs["fcf_W2"])
    L_f2 = np.zeros((128, 4 * 128), np.float32)
    for h in range(2):
        for k in range(2):
            L_f2[:, (2 * h + k) * 128:(2 * h + k + 1) * 128] = \
                W2.T[k * 128:(k + 1) * 128, h * 128:(h + 1) * 128]

    com = {
        "feats_row": _bf(feats.reshape(1, 768)),
        "L_Wg1": _bf(L_Wg1), "L_Wg2": _bf(L_Wg2), "L_nWk": _bf(L_nWk),
        "L_Wv": _bf(L_Wv), "L_G1Q": _bf(L_G1Q),
        "L_G1P8": _f8(L_G1P8),
        "L_PsI8": _f8(L_PsI8.reshape(NB * 4, 128, 256)),
        "sp3": _bf(sp3), "sp1": _bf(sp1), "vec_pack": _f32(vec_pack),
        "L_enc": _bf(_f32(inputs["enc_W"])[:, 0:1].T),
        "L_em1": _bf(np.stack([_f32(inputs["em_W1"][j]).T for j in range(NF)])),
        "L_em2": _bf(np.stack([_f32(inputs["em_W2"][j]).T for j in range(NF)])),
        "L_f1": _bf(W1f.T.reshape(128, 2, 128).transpose(1, 0, 2)),
        "L_f2": _bf(L_f2),
    }

    in_maps = []
    for cix in range(8):
        b, qo = cix // 4, (cix % 4) * 96
        xk = xyz[b].T                      # [3, 384]
        S4 = np.zeros((4, NCH, 128), np.float32)
        for c in range(NCH):
            for qr in range(4):
                qg = qo + 4 * c + qr
                for j in range(3):
                    for t in range(10):
                        col = 32 * qr + 10 * j + t
                        s = np.float32(4.0 * FREQS[t % 5] / TWO_PI)
                        off = np.float32(0.25 if t >= 5 else 0.0)
                        S4[j, c, col] = s
                        S4[3, c, col] = (off + 512.0
                                         - s * np.float32(xyz[b, qg, j]))
        m = dict(com)
        m["xk_f"] = _f32(xk)
        m["s4_f"] = _f32(S4.reshape(4, NCH * 128))
        m["xk_b"] = _bf(xk)
        m["xq_b"] = _bf(xk[:, qo:qo + 96])
        m["feats_b"] = _bf(feats[b].reshape(1, 384))
        m["feats_q"] = _bf(feats[b, qo:qo + 96].reshape(1, 96))
        in_maps.append(m)
    return in_maps


def kernel(**inputs):
    from concourse.bass_utils import run_bass_kernel_spmd

    nc = _build()
    in_maps = _prep_inputs(inputs)
    res = run_bass_kernel_spmd(nc, in_maps, list(range(8)))
    return np.asarray(res.results[0]["out"], np.float32)


if __name__ == "__main__":
    print("smoke build only")


# revision 41
# speedup vs baseline: 1.0096x; 1.0002x over previous
"""NePuEncoder Bass/Tile kernel for 8 Trainium2 NeuronCores.

Sharding: query-parallel. Core c handles batch b=c//4, queries qo=(c%4)*96 ..
qo+96 of that batch. Channel-major layout [128 chan, keys] throughout.

Per-query attention fully fused in SBUF/PSUM:
  - PE:   hpre = G1P8(fp8,K=32)@trig8 + Wg1(bf16)@EK;  logits = Wg2@h;
          pos  = ONE fp8 DoubleRow matmul (I8 (x) VK8  +  PsShift8 (x) trig8)
          (per-query PE cost 560ns vs 800ns all-bf16)
  - relu(hpre+QB[m]): 1/3 on ACT, 2/3 on DVE (balance)
  - ACT:  w = exp(logits + bg2) with accum -> S0
  - DVE:  S1[m] = sum_n (pos + QP[m]) * w   (scalar_tensor_tensor)
Trig features are fp8 e4m3, built in stage 1: r = S4@[xk;1] (fp32 matmul),
round via +/-C trick (DVE), trig = Sin(2*pi*frac) written straight to fp8.
BN rsqrt via fast-inverse-sqrt bit hack + 2 Newton steps (keeps the ACT
exp table resident; no LoadActFuncSet churn).  The per-block AllGather is
split A(80 queries)/B(16+stats, stats ride as f32 bits in the bf16
payload); chunk A's collective+gather hide under the remaining queries.
BN affines are folded into the MLP/FC weights (per-channel scale on lhsT,
constant shift cancels in the next BN), so no full-width affine sits on
the boundary critical path.  Final FC runs on the gathered o2 directly.

HW-validated constraints: GPSIMD does no compute here (TensorScalarPtr /
PSUM access unsupported); tc.If branches contain only ACT ops (DVE ops
inside If/Else crash the device); DMAs issue on sync/gpsimd queues only;
AluOpType.mod is not valid TensorScalar ISA.
"""
import sys

sys.path.insert(0, "/opt/trn_rl_repo")

import numpy as np
import ml_dtypes

B, N, D, DS, LAT, FD, NF = 2, 384, 128, 3, 256, 1, 2
NB = NF + 1
NQ = 96                  # queries per core
NCH = 24                 # stage-1 chunks (4 queries each)
QA = 80                  # chunk-A query count for the split collective
FREQS = np.linspace(1.0, 32.0, 5).astype(np.float64)
EPS = 1e-5
TWO_PI = float(2 * np.pi)
MAGIC = np.uint32(0x5F3759DF)
C_ROUND = float(3 << 22)  # fp32 round-to-nearest-even trick

BF = ml_dtypes.bfloat16
F8NP = ml_dtypes.float8_e4m3


def _bf(x):
    return np.ascontiguousarray(np.asarray(x, np.float32).astype(BF))


def _f8(x):
    return np.ascontiguousarray(np.asarray(x, np.float32).astype(F8NP))


def _f32(x):
    return np.ascontiguousarray(np.asarray(x, np.float32))


def _wpe_split(Wpe):
    """Ws2 [128,30] trig cols with device sign (-sin trick) folded in:
    row r=10j+t: t<5 -> +Wpe[:,3+6t+j] (sin), t>=5 -> -Wpe[:,3+6(t-5)+3+j]."""
    Ws2 = np.zeros((D, 30), np.float32)
    for j in range(3):
        for t in range(10):
            r = 10 * j + t
            if t < 5:
                Ws2[:, r] = -Wpe[:, 3 + 6 * t + j]
            else:
                Ws2[:, r] = Wpe[:, 3 + 6 * (t - 5) + 3 + j]
    return Ws2, Wpe[:, 0:3].astype(np.float32)


_CACHE = {}


def _build(variant="spmd"):
    if variant in _CACHE:
        return _CACHE[variant]

    import concourse.bacc as bacc
    import concourse.bass as bass
    import concourse.tile as tile
    from concourse import mybir

    F32, BF16 = mybir.dt.float32, mybir.dt.bfloat16
    F8 = mybir.dt.float8e4
    U32 = mybir.dt.uint32
    AF = mybir.ActivationFunctionType
    OP = mybir.AluOpType
    PM = mybir.MatmulPerfMode

    nc = bacc.Bacc(None, target_bir_lowering=False,
                   num_devices=(8 if variant == "spmd" else 1))

    def din(name, shape, dt=BF16):
        return nc.dram_tensor(name, shape, dt, kind="ExternalInput")

    # per-core inputs
    xk_f = din("xk_f", [3, 384], F32)
    s4_f = din("s4_f", [4, NCH * 128], F32)
    xk_b = din("xk_b", [3, 384])
    xq_b = din("xq_b", [3, 96])
    feats_b = din("feats_b", [1, 384])
    feats_q = din("feats_q", [1, 96])
    # replicated inputs
    feats_row = din("feats_row", [1, 768])
    L_Wg1 = din("L_Wg1", [NB, 128, 128])
    L_Wg2 = din("L_Wg2", [NB, 128, 128])
    L_nWk = din("L_nWk", [NB, 128, 128])
    L_Wv = din("L_Wv", [NB, 128, 128])
    L_G1Q = din("L_G1Q", [NB, 128, 128])
    L_G1P8 = din("L_G1P8", [NB, 128, 128], F8)
    L_PsI8 = din("L_PsI8", [NB * 4, 128, 256], F8)
    sp3 = din("sp3", [3, NB * 3 * 128])       # nPd4 | G1Pd4 | Pd4 per block
    sp1 = din("sp1", [1, NB * 2 * 128])       # c1 | bpe per block
    vec_pack = din("vec_pack", [128, 24], F32)
    L_enc = din("L_enc", [1, 128])
    L_em1 = din("L_em1", [NF, 128, 128])
    L_em2 = din("L_em2", [NF, 128, 128])
    L_f1 = din("L_f1", [2, 128, 128])
    L_f2 = din("L_f2", [128, 4 * 128])        # f2[h,k] at col (2h+k)*128

    out_d = nc.dram_tensor("out", [2, 256], F32, kind="ExternalOutput")
    RG = [[0, 1, 2, 3, 4, 5, 6, 7]]

    with tile.TileContext(nc) as tc:
        with (
            tc.tile_pool(name="sing", bufs=1) as sing,
            tc.tile_pool(name="fpool", bufs=2) as fpool,
            tc.tile_pool(name="blk", bufs=2) as blk,
            tc.tile_pool(name="hp", bufs=6) as hp,
            tc.tile_pool(name="wp", bufs=6) as wp,
            tc.tile_pool(name="st1", bufs=2) as st1,
            tc.tile_pool(name="wide", bufs=2) as wide,
            tc.tile_pool(name="smalls", bufs=4) as smalls,
            tc.tile_pool(name="ps_a", bufs=2, space="PSUM") as ps_a,
            tc.tile_pool(name="ps_b", bufs=2, space="PSUM") as ps_b,
            tc.tile_pool(name="ps_g", bufs=4, space="PSUM") as ps_g,
            tc.tile_pool(name="dram", bufs=1, space="DRAM") as dram,
        ):
            def load(src, shape, dt=BF16, pool=sing, tag=None, q=None):
                t = pool.tile(shape, dt, tag=tag, name=tag or "ld")
                (q or nc.sync).dma_start(out=t, in_=src)
                return t

            def loadfam(srcT, nblk, tag, dt=BF16, w=128, q=None):
                t = sing.tile([128, nblk * w], dt, tag=tag, name=tag)
                ap = srcT[:]
                s = bass.AP(tensor=ap.tensor, offset=ap.offset,
                            ap=[[w, 128], [128 * w, nblk], [1, w]])
                (q or nc.sync).dma_start(
                    out=t.rearrange("p (i c) -> p i c", i=nblk), in_=s)
                return [t[:, i * w:(i + 1) * w] for i in range(nblk)]

            # ---- critical loads for stage 1 ----
            xko = sing.tile([4, 384], F32, tag="xko")
            nc.vector.memset(xko, 1.0)
            nc.sync.dma_start(out=xko[0:3, :], in_=xk_f[:])
            s4_sb = load(s4_f[:], [4, NCH * 128], F32, tag="s4")

            vp = load(vec_pack[:], [128, 24], F32, tag="vp")
            magic = vp[:, 22:23]

            # fp8 mega tile: [VK8 | 24 trig chunks] each 384 cols
            T8 = sing.tile([128, (1 + NCH) * 384], F8, tag="T8", name="T8")
            t8ap = T8[:]

            def t8_slot(c):
                return T8[:, (1 + c) * 384:(2 + c) * 384]

            # ---------- stage 1: trig via S4 matmul + round + Sin -------
            # even chunks round on DVE (frac), odd on ACT+DVE (-frac, sign
            # folded into the Sin scale)
            cC = sing.tile([128, 1], F32, tag="cC")
            nc.vector.memset(cC, C_ROUND)
            for c in range(NCH):
                rp = ps_a.tile([128, 512], F32, tag="pa")
                nc.tensor.matmul(rp[:, 0:384], s4_sb[:, c * 128:(c + 1) * 128],
                                 xko, start=True, stop=True)
                if c % 2 == 0:
                    n_t = st1.tile([128, 384], F32, tag="nt")
                    nc.vector.tensor_scalar(out=n_t, in0=rp[:, 0:384],
                                            scalar1=C_ROUND, scalar2=C_ROUND,
                                            op0=OP.add, op1=OP.subtract)
                    n_s = st1.tile([128, 384], F32, tag="ns")
                    nc.vector.tensor_tensor(out=n_s, in0=rp[:, 0:384],
                                            in1=n_t, op=OP.subtract)
                    nc.scalar.activation(out=t8_slot(c), in_=n_s, func=AF.Sin,
                                         bias=0.0, scale=TWO_PI)
                else:
                    rc = st1.tile([128, 384], F32, tag="rc")
                    nc.scalar.activation(out=rc, in_=rp[:, 0:384],
                                         func=AF.Identity, bias=cC, scale=1.0)
                    n_s = st1.tile([128, 384], F32, tag="ns")
                    nc.vector.scalar_tensor_tensor(
                        out=n_s, in0=rc, scalar=C_ROUND, in1=rp[:, 0:384],
                        op0=OP.subtract, op1=OP.subtract)
                    nc.scalar.activation(out=t8_slot(c), in_=n_s, func=AF.Sin,
                                         bias=0.0, scale=-TWO_PI)

            # ---- bulk loads (queue behind stage-1 issues) ----
            W = {}
            W['xkb'] = load(xk_b[:], [3, 384], tag="sxkb")
            W['xqb'] = load(xq_b[:], [3, 96], tag="sxqb")
            W['featsb'] = load(feats_b[:], [1, 384], tag="sfb")
            W['featsq'] = load(feats_q[:], [1, 96], tag="sfq")
            W['Wg1'] = loadfam(L_Wg1, NB, "wg1")
            W['Wg2'] = loadfam(L_Wg2, NB, "wg2")
            W['nWk'] = loadfam(L_nWk, NB, "nwk")
            W['Wv'] = loadfam(L_Wv, NB, "wv", q=nc.gpsimd)
            W['G1Q'] = loadfam(L_G1Q, NB, "g1q", q=nc.gpsimd)
            W['G1P8'] = loadfam(L_G1P8, NB, "g1p8", dt=F8, q=nc.gpsimd)
            psi = loadfam(L_PsI8, NB * 4, "psi8", dt=F8, w=256, q=nc.gpsimd)
            W['PsI8'] = [[psi[4 * i + s].rearrange("p (s2 k) -> p s2 k", s2=2)
                          for s in range(4)] for i in range(NB)]
            sp3_sb = load(sp3[:], [3, NB * 3 * 128], tag="sp3", q=nc.gpsimd)
            sp1_sb = load(sp1[:], [1, NB * 2 * 128], tag="sp1", q=nc.gpsimd)
            W['enc'] = load(L_enc[:], [1, 128], tag="enc", q=nc.gpsimd)
            W['em1'] = loadfam(L_em1, NF, "em1", q=nc.gpsimd)
            W['em2'] = loadfam(L_em2, NF, "em2", q=nc.gpsimd)
            W['f1'] = loadfam(L_f1, 2, "f1", q=nc.gpsimd)
            f2all = load(L_f2[:], [128, 4 * 128], tag="f2", q=nc.gpsimd)
            W['f2'] = [[f2all[:, (2 * h + k) * 128:(2 * h + k + 1) * 128]
                        for k in range(2)] for h in range(2)]

            def blkslice(base, i, j, w3):
                return base[:, (i * 3 + j) * 128:(i * 3 + j) * 128 + 128]

            W['nPd4'] = [sp3_sb[:, (i * 3 + 0) * 128:(i * 3 + 0) * 128 + 128]
                         for i in range(NB)]
            W['G1Pd4'] = [sp3_sb[:, (i * 3 + 1) * 128:(i * 3 + 1) * 128 + 128]
                          for i in range(NB)]
            W['Pd4'] = [sp3_sb[:, (i * 3 + 2) * 128:(i * 3 + 2) * 128 + 128]
                        for i in range(NB)]
            W['c1'] = [sp1_sb[:, (2 * i) * 128:(2 * i) * 128 + 128]
                       for i in range(NB)]
            W['bpe'] = [sp1_sb[:, (2 * i + 1) * 128:(2 * i + 1) * 128 + 128]
                        for i in range(NB)]
            W['bg2'] = [vp[:, i:i + 1] for i in range(NB)]
            W['gam'] = [vp[:, 3 + i:4 + i] for i in range(NB)]
            W['bet'] = [vp[:, 6 + i:7 + i] for i in range(NB)]
            W['encb'] = vp[:, 9:10]
            W['emb1'] = [vp[:, 10 + j:11 + j] for j in range(NF)]
            W['emb2'] = [vp[:, 12 + j:13 + j] for j in range(NF)]
            W['emg'] = [vp[:, 14 + j:15 + j] for j in range(NF)]
            W['embe'] = [vp[:, 16 + j:17 + j] for j in range(NF)]
            W['f1b'] = [vp[:, 18 + h:19 + h] for h in range(2)]
            W['f2b'] = [vp[:, 20 + h:21 + h] for h in range(2)]
            magic = vp[:, 22:23]

            ones96 = sing.tile([1, 96], BF16, tag="ones96")
            nc.vector.memset(ones96, 1.0)
            ones384 = sing.tile([128, 384], BF16, tag="ones384")
            nc.vector.memset(ones384, 1.0)
            dumA = sing.tile([128, 1], BF16, tag="dumA")

            # ---------- initial features (ps_b: overlaps stage-1) ----------
            fb = fpool.tile([128, 384], BF16, tag="fb")
            p = ps_b.tile([128, 512], F32, tag="pb")
            nc.tensor.matmul(p[:, 0:384], W['enc'], W['featsb'], start=True,
                             stop=True)
            nc.scalar.activation(out=fb, in_=p[:, 0:384], func=AF.Identity,
                                 bias=W['encb'], scale=1.0)
            fq = fpool.tile([128, 96], BF16, tag="fq")
            p = ps_b.tile([128, 512], F32, tag="pb")
            nc.tensor.matmul(p[:, 0:96], W['enc'], W['featsq'], start=True,
                             stop=True)
            nc.scalar.activation(out=fq, in_=p[:, 0:96], func=AF.Identity,
                                 bias=W['encb'], scale=1.0)

            pid = nc.scalar.partition_id()

            def rsqrt(var_ap, tag):
                """y ~ 1/sqrt(var + EPS): bit hack + 2 Newton steps (DVE)."""
                u = smalls.tile([128, 1], F32, tag="rq_u")
                nc.vector.tensor_scalar(out=u, in0=var_ap, scalar1=EPS,
                                        scalar2=None, op0=OP.add)
                h = smalls.tile([128, 1], U32, tag="rq_h")
                nc.vector.tensor_scalar(out=h, in0=u[:].bitcast(U32),
                                        scalar1=1, scalar2=None,
                                        op0=OP.logical_shift_right)
                y = smalls.tile([128, 1], F32, tag=tag, name=tag)
                nc.vector.tensor_tensor(out=y[:].bitcast(U32),
                                        in0=magic.bitcast(U32), in1=h,
                                        op=OP.subtract)
                for _ in range(2):
                    t1 = smalls.tile([128, 1], F32, tag="rq_t1")
                    nc.vector.tensor_tensor(out=t1, in0=y, in1=y, op=OP.mult)
                    nc.vector.tensor_tensor(out=t1, in0=u, in1=t1, op=OP.mult)
                    nc.vector.tensor_scalar(out=t1, in0=t1, scalar1=-0.5,
                                            scalar2=1.5, op0=OP.mult,
                                            op1=OP.add)
                    y2 = smalls.tile([128, 1], F32, tag=tag, name=tag)
                    nc.vector.tensor_tensor(out=y2, in0=y, in1=t1, op=OP.mult)
                    y = y2
                return y

            # ---------- transformer blocks ----------
            for i in range(NB):
                # block consts: EK(bf16), VK8(fp8), QB, QP  (ps_b ring)
                pa = ps_b.tile([128, 512], F32, tag="pb")
                nc.tensor.matmul(pa[:, 0:384], W['nWk'][i], fb, start=True,
                                 stop=False)
                nc.tensor.matmul(pa[:, 0:384], W['nPd4'][i][0:3, :], W['xkb'],
                                 start=False, stop=True)
                EK = blk.tile([128, 384], BF16, tag="EK")
                nc.scalar.copy(EK, pa[:, 0:384])

                pb = ps_b.tile([128, 512], F32, tag="pb")
                nc.tensor.matmul(pb[:, 0:384], W['Wv'][i], fb, start=True,
                                 stop=False)
                nc.tensor.matmul(pb[:, 0:384], W['nPd4'][i][0:3, :], W['xkb'],
                                 start=False, stop=True)
                nc.scalar.copy(T8[:, 0:384], pb[:, 0:384])   # VK8 fp8

                pa = ps_b.tile([128, 512], F32, tag="pb")
                nc.tensor.matmul(pa[:, 0:96], W['G1Q'][i], fq, start=True,
                                 stop=False)
                nc.tensor.matmul(pa[:, 0:96], W['G1Pd4'][i][0:3, :], W['xqb'],
                                 start=False, stop=False)
                nc.tensor.matmul(pa[:, 0:96], W['c1'][i][0:1, :], ones96,
                                 start=False, stop=True)
                QB = blk.tile([128, 96], F32, tag="QB")
                nc.vector.tensor_copy(QB, pa[:, 0:96])

                pb = ps_b.tile([128, 512], F32, tag="pb")
                nc.tensor.matmul(pb[:, 0:96], W['Pd4'][i][0:3, :], W['xqb'],
                                 start=True, stop=False)
                nc.tensor.matmul(pb[:, 0:96], W['bpe'][i][0:1, :], ones96,
                                 start=False, stop=True)
                QP = blk.tile([128, 96], F32, tag="QP")
                nc.vector.tensor_copy(QP, pb[:, 0:96])

                S1 = blk.tile([128, 96], F32, tag="S1")
                S0 = blk.tile([128, 96], F32, tag="S0")
                # payload: 96 bf16 o-cols + 2 f32 stats (as raw bits)
                P16 = blk.tile([128, 100], BF16, tag="P16")
                P32 = P16[:].bitcast(F32)          # [128, 50]

                ag_in_A = dram.tile([128, QA], BF16, tag=f"aginA{i}")
                ag_in_B = dram.tile([128, 100 - QA], BF16, tag=f"aginB{i}")
                shr = dict(addr_space="Shared") if variant == "spmd" else {}
                ag_out_A = dram.tile([8, 128, QA], BF16, tag=f"agoutA{i}",
                                     **shr)
                ag_out_B = dram.tile([8, 128, 100 - QA], BF16,
                                     tag=f"agoutB{i}", **shr)

                def fan(ag_in, ag_out, src_ap, queues):
                    nc.sync.dma_start(out=ag_in, in_=src_ap)
                    if variant == "spmd":
                        nc.gpsimd.collective_compute(
                            "AllGather", OP.bypass, replica_groups=RG,
                            ins=[ag_in[:].opt()], outs=[ag_out[:].opt()])
                    else:
                        for cc in range(8):
                            queues[cc % len(queues)].dma_start(
                                out=ag_out[cc], in_=src_ap)

                # gathered tile: [p, core, 100] bf16 (o cols 0:96 + stat bits)
                o_ext = wide.tile([128, 8, 100], BF16, tag="oext")
                oe = o_ext[:]
                oe32 = oe.bitcast(F32)             # [p, 8, 50]

                def o_half(h):
                    """[128, 384] bf16 view: cores 4h..4h+4, o cols 0:96."""
                    return o_ext[:, 4 * h:4 * h + 4, 0:96]

                # attention, 1-query software pipeline: relu(m+1) is
                # emitted before stt(m) so DVE never head-of-line blocks;
                # relu 1/3 ACT 2/3 DVE, exp+accum ACT, S1 stt DVE, fp8 DR pos
                def hpre_relu(m):
                    c, s = m // 4, m % 4
                    tsl8 = T8[32 * s:32 * s + 32,
                              (1 + c) * 384:(1 + c) * 384 + 384]
                    pa = ps_a.tile([128, 512], F32, tag="pa")
                    nc.tensor.matmul(pa[:, 0:384],
                                     W['G1P8'][i][32 * s:32 * s + 32, :],
                                     tsl8, start=True, stop=False,
                                     tile_position=(32 * s, 0))
                    nc.tensor.matmul(pa[:, 0:384], W['Wg1'][i], EK,
                                     start=False, stop=True)
                    h_t = hp.tile([128, 384], BF16, tag="h")
                    if m % 3 == 0:
                        nc.scalar.activation(out=h_t, in_=pa[:, 0:384],
                                             func=AF.Relu,
                                             bias=QB[:, m:m + 1], scale=1.0)
                    else:
                        nc.vector.tensor_scalar(
                            out=h_t, in0=pa[:, 0:384],
                            scalar1=QB[:, m:m + 1],
                            scalar2=0.0, op0=OP.add, op1=OP.max)
                    return h_t

                hq = [hpre_relu(0), hpre_relu(1)]
                for m in range(NQ):
                    c, s = m // 4, m % 4
                    lg = ps_g.tile([128, 512], F32, tag="lg")
                    nc.tensor.matmul(lg[:, 0:384], W['Wg2'][i], hq.pop(0),
                                     start=True, stop=True)
                    if m + 2 < NQ:
                        hq.append(hpre_relu(m + 2))
                    w_t = wp.tile([128, 384], BF16, tag="w")
                    nc.scalar.activation(out=w_t, in_=lg[:, 0:384],
                                         func=AF.Exp, bias=W['bg2'][i],
                                         scale=1.0, accum_out=S0[:, m:m + 1])
                    if m >= NQ - 3:
                        pos = ps_a.tile([128, 512], F32, tag="pa")
                    else:
                        pos = ps_b.tile([128, 512], F32, tag="pb")
                    rhs = bass.AP(tensor=t8ap.tensor, offset=t8ap.offset,
                                  ap=[list(t8ap.ap[0]), [(1 + c) * 384, 2],
                                      [1, 384]])
                    nc.tensor.matmul(pos[:, 0:384], W['PsI8'][i][s], rhs,
                                     start=True, stop=True,
                                     perf_mode=PM.DoubleRow)
                    nc.vector.scalar_tensor_tensor(
                        out=dumA.broadcast_to((128, 384)),
                        in0=pos[:, 0:384], scalar=QP[:, m:m + 1],
                        in1=w_t, op0=OP.add, op1=OP.mult,
                        accum_out=S1[:, m:m + 1])

                    if m == QA + 3:
                        # chunk A payload + collective (hidden under B's
                        # attention)
                        R = smalls.tile([128, QA], F32, tag="RA")
                        nc.vector.reciprocal(out=R, in_=S0[:, 0:QA])
                        nc.vector.tensor_tensor(out=R, in0=S1[:, 0:QA],
                                                in1=R, op=OP.mult)
                        nc.vector.tensor_tensor(out=P16[:, 0:QA], in0=R,
                                                in1=fq[:, 0:QA], op=OP.add)
                        fan(ag_in_A, ag_out_A, P16[:, 0:QA], [nc.sync])
                        src = bass.AP(
                            tensor=ag_out_A[:].tensor,
                            offset=ag_out_A[:].offset,
                            ap=[[QA, 128], [128 * QA, 8], [1, QA]])
                        nc.sync.dma_start(out=o_ext[:, :, 0:QA], in_=src)

                # chunk B payload + stats
                R = smalls.tile([128, 16], F32, tag="RB")
                nc.vector.reciprocal(out=R, in_=S0[:, QA:96])
                nc.vector.tensor_tensor(out=R, in0=S1[:, QA:96], in1=R,
                                        op=OP.mult)
                nc.vector.tensor_tensor(out=P16[:, QA:96], in0=R,
                                        in1=fq[:, QA:96], op=OP.add)
                st = smalls.tile([128, 6], F32, tag="bnst")
                nc.vector.bn_stats(out=st, in_=P16[:, 0:96])
                mv = smalls.tile([128, 2], F32, tag="bnmv")
                nc.vector.bn_aggr(out=mv, in_=st)
                nc.vector.tensor_copy(P32[:, 48:49], mv[:, 0:1])
                msq = smalls.tile([128, 1], F32, tag="msq")
                nc.vector.tensor_tensor(out=msq, in0=mv[:, 0:1],
                                        in1=mv[:, 0:1], op=OP.mult)
                nc.vector.tensor_tensor(out=P32[:, 49:50], in0=mv[:, 1:2],
                                        in1=msq, op=OP.add)
                fan(ag_in_B, ag_out_B, P16[:, QA:100],
                    [nc.sync, nc.gpsimd, nc.sync, nc.gpsimd, nc.sync,
                     nc.gpsimd, nc.sync, nc.sync])
                srcB = bass.AP(tensor=ag_out_B[:].tensor,
                               offset=ag_out_B[:].offset,
                               ap=[[100 - QA, 128], [128 * (100 - QA), 8],
                                   [1, 100 - QA]])
                nc.sync.dma_start(out=o_ext[:, :, QA:100], in_=srcB)

                mg = smalls.tile([128, 1], F32, tag="mg")
                stm = bass.AP(tensor=oe32.tensor, offset=oe32.offset + 48,
                              ap=[list(oe32.ap[0]), [50, 8]])
                nc.vector.tensor_reduce(out=mg, in_=stm,
                                        axis=mybir.AxisListType.X, op=OP.add)
                nc.vector.tensor_scalar(out=mg, in0=mg, scalar1=0.125,
                                        scalar2=None, op0=OP.mult)
                e2g = smalls.tile([128, 1], F32, tag="e2g")
                ste = bass.AP(tensor=oe32.tensor, offset=oe32.offset + 49,
                              ap=[list(oe32.ap[0]), [50, 8]])
                nc.vector.tensor_reduce(out=e2g, in_=ste,
                                        axis=mybir.AxisListType.X, op=OP.add)
                nc.vector.tensor_scalar(out=e2g, in0=e2g, scalar1=0.125,
                                        scalar2=None, op0=OP.mult)
                var = smalls.tile([128, 1], F32, tag="var")
                nc.vector.tensor_tensor(out=var, in0=mg, in1=mg, op=OP.mult)
                nc.vector.tensor_tensor(out=var, in0=e2g, in1=var,
                                        op=OP.subtract)
                rs = rsqrt(var, "rs")
                sc = smalls.tile([128, 1], F32, tag="sc")
                nc.vector.tensor_tensor(out=sc, in0=W['gam'][i], in1=rs,
                                        op=OP.mult)
                b2 = smalls.tile([128, 1], F32, tag="b2")
                nc.vector.tensor_scalar(out=b2, in0=mg, scalar1=sc,
                                        scalar2=None, op0=OP.mult)
                nc.vector.tensor_tensor(out=b2, in0=W['bet'][i], in1=b2,
                                        op=OP.subtract)

                if i < NB - 1:
                    fq = fpool.tile([128, 96], BF16, tag="fq")
                    nc.vector.tensor_scalar(out=fq, in0=P16[:, 0:96],
                                            scalar1=sc, scalar2=b2,
                                            op0=OP.mult, op1=OP.add)
                if i == NB - 1:
                    fb = None                     # last block: no next EK/VK
                else:
                    fb = fpool.tile([128, 384], BF16, tag="fb")
                    with tc.If(pid < 4) as cmp:
                        nc.scalar.activation(out=fb, in_=o_half(0),
                                             func=AF.Identity, bias=b2,
                                             scale=sc)
                    with cmp.Else():
                        nc.scalar.activation(out=fb, in_=o_half(1),
                                             func=AF.Identity, bias=b2,
                                             scale=sc)

                # ---------- MLP with BN1 affine folded into layer 1 ----------
                if i > 0:
                    j = i - 1
                    # W1' = W1 * diag(sc)  (scale lhsT rows);  bias1' =
                    # W1 @ b2 + em_b1.  The +b2 shift of the residual input
                    # cancels inside BN2, so o2' = sc*o + y2 suffices.
                    em1s = wide.tile([128, 128], BF16, tag="em1s")
                    nc.vector.tensor_scalar(out=em1s, in0=W['em1'][j],
                                            scalar1=sc, scalar2=None,
                                            op0=OP.mult)
                    b2b = smalls.tile([128, 1], BF16, tag="b2b")
                    nc.vector.tensor_copy(b2b, b2)
                    pb1 = ps_b.tile([128, 512], F32, tag="pb")
                    nc.tensor.matmul(pb1[:, 0:1], W['em1'][j], b2b,
                                     start=True, stop=True)
                    bias1 = smalls.tile([128, 1], F32, tag="bias1")
                    nc.vector.tensor_tensor(out=bias1, in0=pb1[:, 0:1],
                                            in1=W['emb1'][j], op=OP.add)

                    def mlp_layer(lw, bias_ap, xins, width, tag):
                        t = wide.tile([128, width], BF16, tag=tag)
                        for hh, xin in enumerate(xins):
                            wdt = min(384, width - hh * 384)
                            pp = ps_a.tile([128, 512], F32, tag="pa")
                            nc.tensor.matmul(pp[:, 0:wdt], lw, xin,
                                             start=True, stop=True)
                            nc.scalar.activation(
                                out=t[:, hh * 384:hh * 384 + wdt],
                                in_=pp[:, 0:wdt], func=AF.Relu,
                                bias=bias_ap, scale=1.0)
                        return t

                    y1f = mlp_layer(em1s, bias1, [o_half(0), o_half(1)],
                                    768, "y1f")
                    y2f = mlp_layer(W['em2'][j], W['emb2'][j],
                                    [y1f[:, 0:384], y1f[:, 384:768]],
                                    768, "y2f")
                    o2f = wide.tile([128, 768], BF16, tag="o2f")
                    nc.vector.scalar_tensor_tensor(
                        out=o2f[:, 0:384], in0=o_half(0), scalar=sc,
                        in1=y2f[:, 0:384], op0=OP.mult, op1=OP.add)
                    nc.vector.scalar_tensor_tensor(
                        out=o2f[:, 384:768], in0=o_half(1), scalar=sc,
                        in1=y2f[:, 384:768], op0=OP.mult, op1=OP.add)
                    if i < NB - 1:
                        y1q = mlp_layer(em1s, bias1, [P16[:, 0:96]], 96,
                                        "y1q")
                        y2q = mlp_layer(W['em2'][j], W['emb2'][j], [y1q], 96,
                                        "y2q")
                        o2q = wide.tile([128, 96], BF16, tag="o2q")
                        nc.vector.scalar_tensor_tensor(
                            out=o2q, in0=P16[:, 0:96], scalar=sc, in1=y2q,
                            op0=OP.mult, op1=OP.add)

                    st2 = smalls.tile([128, 2, 6], F32, tag="st2")
                    nc.vector.bn_stats(out=st2[:, 0, :], in_=o2f[:, 0:384])
                    nc.vector.bn_stats(out=st2[:, 1, :], in_=o2f[:, 384:768])
                    mv2 = smalls.tile([128, 2], F32, tag="mv2")
                    nc.vector.bn_aggr(out=mv2, in_=st2)
                    rs2 = rsqrt(mv2[:, 1:2], "rs2")
                    sc2 = smalls.tile([128, 1], F32, tag="sc")
                    nc.vector.tensor_tensor(out=sc2, in0=W['emg'][j], in1=rs2,
                                            op=OP.mult)
                    b22 = smalls.tile([128, 1], F32, tag="b2")
                    nc.vector.tensor_scalar(out=b22, in0=mv2[:, 0:1],
                                            scalar1=sc2, scalar2=None,
                                            op0=OP.mult)
                    nc.vector.tensor_tensor(out=b22, in0=W['embe'][j],
                                            in1=b22, op=OP.subtract)
                    if i < NB - 1:
                        fq = fpool.tile([128, 96], BF16, tag="fq")
                        nc.vector.tensor_scalar(out=fq, in0=o2q, scalar1=sc2,
                                                scalar2=b22, op0=OP.mult,
                                                op1=OP.add)
                    if i < NB - 1:
                        fb = fpool.tile([128, 384], BF16, tag="fb")
                        with tc.If(pid < 4) as cmp:
                            nc.scalar.activation(out=fb, in_=o2f[:, 0:384],
                                                 func=AF.Identity, bias=b22,
                                                 scale=sc2)
                        with cmp.Else():
                            nc.scalar.activation(out=fb, in_=o2f[:, 384:768],
                                                 func=AF.Identity, bias=b22,
                                                 scale=sc2)

            # ---------- final FC + max (BN2 affine folded into f1) ----------
            om = smalls.tile([128, 2, 2], F32, tag="om")
            b22b = smalls.tile([128, 1], BF16, tag="b22b")
            nc.vector.tensor_copy(b22b, b22)
            f1s, fc_b1 = [], []
            for h in range(2):
                t = wide.tile([128, 128], BF16, tag=f"f1s{h}")
                nc.vector.tensor_scalar(out=t, in0=W['f1'][h], scalar1=sc2,
                                        scalar2=None, op0=OP.mult)
                f1s.append(t)
                pb1 = ps_b.tile([128, 512], F32, tag="pb")
                nc.tensor.matmul(pb1[:, 0:1], W['f1'][h], b22b,
                                 start=True, stop=True)
                bb1 = smalls.tile([128, 1], F32, tag=f"fcb{h}")
                nc.vector.tensor_tensor(out=bb1, in0=pb1[:, 0:1],
                                        in1=W['f1b'][h], op=OP.add)
                fc_b1.append(bb1)
            for bb in range(2):
                fbb = o2f[:, bb * 384:(bb + 1) * 384]
                e1 = []
                for h in range(2):
                    pp = ps_a.tile([128, 512], F32, tag="pa")
                    nc.tensor.matmul(pp[:, 0:384], f1s[h], fbb,
                                     start=True, stop=True)
                    e1t = wide.tile([128, 384], BF16, tag=f"e1{h}")
                    if h == 0:
                        nc.scalar.activation(out=e1t, in_=pp[:, 0:384],
                                             func=AF.Relu, bias=fc_b1[h],
                                             scale=1.0)
                    else:
                        nc.vector.tensor_scalar(out=e1t, in0=pp[:, 0:384],
                                                scalar1=fc_b1[h],
                                                scalar2=0.0, op0=OP.add,
                                                op1=OP.max)
                    e1.append(e1t)
                for h in range(2):
                    pp = ps_a.tile([128, 512], F32, tag="pa")
                    nc.tensor.matmul(pp[:, 0:384], W['f2'][h][0], e1[0],
                                     start=True, stop=False)
                    nc.tensor.matmul(pp[:, 0:384], W['f2'][h][1], e1[1],
                                     start=False, stop=True)
                    mx = smalls.tile([128, 1], F32, tag="mx")
                    nc.vector.tensor_reduce(out=mx, in_=pp[:, 0:384],
                                            axis=mybir.AxisListType.X,
                                            op=OP.max)
                    nc.vector.tensor_scalar(out=om[:, bb, h:h + 1], in0=mx,
                                            scalar1=W['f2b'][h], scalar2=None,
                                            op0=OP.add)
            dst = bass.AP(tensor=out_d[:].tensor, offset=out_d[:].offset,
                          ap=[[1, 128], [256, 2], [128, 2]])
            nc.sync.dma_start(out=dst, in_=om)

    nc.compile()
    _CACHE[variant] = nc
    return nc


def _prep_inputs(inputs):
    """Host-side constant relayout + per-core slicing. Returns in_maps list."""
    xyz = _f32(inputs["xyz"])          # [2, 384, 3]
    feats = _f32(inputs["feats"])      # [2, 384, 1]

    Wq, Wk, Wv = inputs["tb_Wq"], inputs["tb_Wk"], inputs["tb_Wv"]
    Wg1, bg1 = inputs["tb_Wg1"], inputs["tb_bg1"]
    Wg2, bg2 = inputs["tb_Wg2"], inputs["tb_bg2"]
    Wpe, bpe = inputs["tb_Wpe"], inputs["tb_bpe"]

    L_nWk = np.zeros((NB, 128, 128), np.float32)
    L_Wv = np.zeros((NB, 128, 128), np.float32)
    L_G1Q = np.zeros((NB, 128, 128), np.float32)
    L_Wg1 = np.zeros((NB, 128, 128), np.float32)
    L_Wg2 = np.zeros((NB, 128, 128), np.float32)
    L_G1P8 = np.zeros((NB, 128, 128), np.float32)
    L_PsI8 = np.zeros((NB, 4, 128, 256), np.float32)
    sp3 = np.zeros((3, NB * 3 * 128), np.float32)
    sp1 = np.zeros((1, NB * 2 * 128), np.float32)
    I128 = np.eye(128, dtype=np.float32)
    for i in range(NB):
        Ws2, Wd = _wpe_split(_f32(Wpe[i]))
        g1 = _f32(Wg1[i])
        L_nWk[i] = (-_f32(Wk[i])).T
        L_Wv[i] = _f32(Wv[i]).T
        L_G1Q[i] = (g1 @ _f32(Wq[i])).T
        L_Wg1[i] = g1.T
        L_Wg2[i] = _f32(Wg2[i]).T
        G1P = (g1 @ Ws2).T                      # [30, 128]
        PsT = Ws2.T                             # [30, 128]
        for s in range(4):
            L_G1P8[i, 32 * s:32 * s + 30, :] = G1P
            L_PsI8[i, s, :, 0:128] = I128
            L_PsI8[i, s, 32 * s:32 * s + 30, 128:256] = PsT
        sp3[:, (i * 3 + 0) * 128:(i * 3 + 0) * 128 + 128] = (-4.0 * Wd).T
        sp3[:, (i * 3 + 1) * 128:(i * 3 + 1) * 128 + 128] = (4.0 * (g1 @ Wd)).T
        sp3[:, (i * 3 + 2) * 128:(i * 3 + 2) * 128 + 128] = (4.0 * Wd).T
        sp1[0, (2 * i) * 128:(2 * i) * 128 + 128] = g1 @ _f32(bpe[i]) + _f32(bg1[i])
        sp1[0, (2 * i + 1) * 128:(2 * i + 1) * 128 + 128] = _f32(bpe[i])

    vec_pack = np.zeros((128, 24), np.float32)
    for i in range(NB):
        vec_pack[:, i] = _f32(bg2[i])
        vec_pack[:, 3 + i] = _f32(inputs["tb_gamma"][i])
        vec_pack[:, 6 + i] = _f32(inputs["tb_beta"][i])
    vec_pack[:, 9] = _f32(inputs["enc_b"])
    for j in range(NF):
        vec_pack[:, 10 + j] = _f32(inputs["em_b1"][j])
        vec_pack[:, 12 + j] = _f32(inputs["em_b2"][j])
        vec_pack[:, 14 + j] = _f32(inputs["em_gamma"][j])
        vec_pack[:, 16 + j] = _f32(inputs["em_beta"][j])
    W1f = _f32(inputs["fcf_W1"])
    vec_pack[:, 18] = _f32(inputs["fcf_b1"])[0:128]
    vec_pack[:, 19] = _f32(inputs["fcf_b1"])[128:256]
    vec_pack[:, 20] = _f32(inputs["fcf_b2"])[0:128]
    vec_pack[:, 21] = _f32(inputs["fcf_b2"])[128:256]
    vec_pack[:, 22] = np.full(128, np.frombuffer(
        MAGIC.tobytes(), dtype=np.float32)[0], np.float32)
    vec_pack[:, 23] = -np.pi

    W2 = _f32(inputs["fcf_W2"])
    L_f2 = np.zeros((128, 4 * 128), np.float32)
    for h in range(2):
        for k in range(2):
            L_f2[:, (2 * h + k) * 128:(2 * h + k + 1) * 128] = \
                W2.T[k * 128:(k + 1) * 128, h * 128:(h + 1) * 128]

    com = {
        "feats_row": _bf(feats.reshape(1, 768)),
        "L_Wg1": _bf(L_Wg1), "L_Wg2": _bf(L_Wg2), "L_nWk": _bf(L_nWk),
        "L_Wv": _bf(L_Wv), "L_G1Q": _bf(L_G1Q),
        "L_G1P8": _f8(L_G1P8),
        "L_PsI8": _f8(L_PsI8.reshape(NB * 4, 128, 256)),
        "sp3": _bf(sp3), "sp1": _bf(sp1), "vec_pack": _f32(vec_pack),
        "L_enc": _bf(_f32(inputs["enc_W"])[:, 0:1].T),
        "L_em1": _bf(np.stack([_f32(inputs["em_W1"][j]).T for j in range(NF)])),
        "L_em2": _bf(np.stack([_f32(inputs["em_W2"][j]).T for j in range(NF)])),
        "L_f1": _bf(W1f.T.reshape(128, 2, 128).transpose(1, 0, 2)),
        "L_f2": _bf(L_f2),
    }

    in_maps = []
    for cix in range(8):
        b, qo = cix // 4, (cix % 4) * 96
        xk = xyz[b].T                      # [3, 384]
        S4 = np.zeros((4, NCH, 128), np.float32)
        for c in range(NCH):
            for qr in range(4):
                qg = qo + 4 * c + qr
                for j in range(3):
                    for t in range(10):
                        col = 32 * qr + 10 * j + t
                        s = np.float32(4.0 * FREQS[t % 5] / TWO_PI)
                        off = np.float32(0.25 if t >= 5 else 0.0)
                        S4[j, c, col] = s
                        S4[3, c, col] = (off + 512.0
                                         - s * np.float32(xyz[b, qg, j]))
        m = dict(com)
        m["xk_f"] = _f32(xk)
        m["s4_f"] = _f32(S4.reshape(4, NCH * 128))
        m["xk_b"] = _bf(xk)
        m["xq_b"] = _bf(xk[:, qo:qo + 96])
        m["feats_b"] = _bf(feats[b].reshape(1, 384))
        m["feats_q"] = _bf(feats[b, qo:qo + 96].reshape(1, 96))
        in_maps.append(m)
    return in_maps


def kernel(**inputs):
    from concourse.bass_utils import run_bass_kernel_spmd

    nc = _build()
    in_maps = _prep_inputs(inputs)
    res = run_bass_kernel_spmd(nc, in_maps, list(range(8)))
    return np.asarray(res.results[0]["out"], np.float32)


if __name__ == "__main__":
    print("smoke build only")
